# revision 49
# baseline (speedup 1.0000x reference)
"""Trainium2 Bass kernel for nn_MixAttention (GAT-style mixed attention).

Strategy (8 cores, i-sharded over query rows, transposed compute):
  - Device computes scores in transposed layout [j on partitions, i free] so
    out^T += hc_chunk.T @ P^T_chunk contracts over partitions, no transposes.
  - lrelu decomposition: lrelu(x) = 0.01x + 0.99*relu(x). For both score
    terms the relu part is computed per entry; the linear part is rank-1:
    the per-i piece cancels in the row softmax, the per-j piece rides along
    as a multiplicative exp(lv[j]) folded into the mask op's scalar slot.
  - Per chunk: tA = relu(bcA + agrid_c) and tB = relu(bcB + bgrid_c) via
    single tensor_scalar(add, max-0) ops (fp16 packed = DVE 4x mode),
    z = tA + tB (fp16 TT, 2x), exp over half-groups of 2 chunks on ACT
    (scale=0.99, per-core bias tensor), then Pm = P * slab (plain mult;
    slab is the uint16 adjacency so masked entries are exactly 0; uint16
    keeps the DVE 2-byte fast path). exp(lv) is folded into the matmul
    stationaries instead: hc' = hc * explv (ACT copy with per-partition
    scale) and the rowsum stationary is an explv bf16 column.
  - PE accumulates out^T += hc'_c @ Pm and rowsum += explv_c @ Pm in bf16.
  - Phase-0 grids (dstA, dstB, sigma, lv) come from per-chunk PE matmuls
    into one PSUM block (129-wide psA with dstA as column F, 2-wide
    sigma/dstB pairs packed into the same banks) post-processed in batched
    [128, 8] ops. psA and sigma/dstB accumulation groups are emitted in
    disjoint time ranges: interleaving two open matmul accumulations in
    one PSUM bank corrupts results, as does issuing two matmuls on
    alternating stationaries with 1-column movings.
  - Phase 0 and phase 1 are software-pipelined: the emission loop issues
    phase-0 group g, then phase-1 groups 2(g-1) and 2(g-1)+1, then the hc
    copies of g, so every engine's in-order queue interleaves both phases.
    Grid tiles are per-group (unique names) to avoid whole-tile WAR
    hazards that would serialize the phases.
  - h inputs are bf16 (host-converted): per-query projection errors cancel
    in the row softmax; per-key errors are ~0.4% and measured end-to-end
    rel err is 4.8e-3 against the fp32 reference.
  - All per-core constants enter via input tensors (negc0/negclv/cabA/
    cabB), so a single compiled program serves every core and input set.
"""

import numpy as np

N = 8192
K = 256
F = 128
NC = 8
S = N // NC          # 1024 query rows per core
NCH = N // 128       # 64 j-chunks
KC = K // 128        # 2 contraction chunks
G0 = 4               # j-chunks per phase-0 stream group (== mask slab)
W0 = G0 * 128
GRP = 4              # j-chunks per exp group / mask slab
NSLAB = NCH // GRP   # 16

_BUILD_CACHE = {}


def _build_program():
    import contextlib

    import concourse.bacc as bacc
    import concourse.tile as tile
    from concourse import mybir

    nc = bacc.Bacc("TRN2", target_bir_lowering=False, debug=False, num_devices=NC)
    dt = mybir.dt
    AF = mybir.ActivationFunctionType
    OP = mybir.AluOpType

    hctxT = nc.dram_tensor("hctxT", [K, N], dt.bfloat16, kind="ExternalInput")
    hstrT = nc.dram_tensor("hstrT", [K, N], dt.bfloat16, kind="ExternalInput")
    hctxT_my = nc.dram_tensor("hctxT_my", [K, S], dt.bfloat16,
                              kind="ExternalInput")
    hstrT_my = nc.dram_tensor("hstrT_my", [K, S], dt.bfloat16,
                              kind="ExternalInput")
    # wub = [wpack (F+1) | uB16 (3) | vA116 (1)] packed: one DMA per k-chunk
    wub = nc.dram_tensor("wub", [K, F + 5], dt.bfloat16, kind="ExternalInput")
    # alternating mask split: even slabs chunks {0,1} uint16 (DVE) /
    # {2,3} uint8 (gpsimd); odd slabs {0} / {1,2,3}
    maskPd = nc.dram_tensor("maskPd", [128, 24 * S], dt.uint16,
                            kind="ExternalInput")
    maskPg = nc.dram_tensor("maskPg", [128, 40 * S], dt.uint8,
                            kind="ExternalInput")
    # columns: negc0 | negclv | cabA | cabB — one DMA
    smalls4 = nc.dram_tensor("smalls4", [128, 4], dt.float32,
                             kind="ExternalInput")
    ident = nc.dram_tensor("ident", [128, 128], dt.float16,
                           kind="ExternalInput")
    outT = nc.dram_tensor("outT", [F, S], dt.float32, kind="ExternalOutput")

    with tile.TileContext(nc) as tc:
        with contextlib.ExitStack() as ctx:
            vecs = ctx.enter_context(tc.tile_pool(name="vecs", bufs=1))
            hcpool = ctx.enter_context(tc.tile_pool(name="hc", bufs=1))
            stp = ctx.enter_context(tc.tile_pool(name="stream", bufs=2))
            work = ctx.enter_context(tc.tile_pool(name="work", bufs=3))
            grpp = ctx.enter_context(tc.tile_pool(name="grp", bufs=3))
            pmp = ctx.enter_context(tc.tile_pool(name="pm", bufs=6))
            slabp = ctx.enter_context(tc.tile_pool(name="slabp", bufs=3))

            # ---- small inputs (3 DMAs + 4 my-slice DMAs) ----
            sm4_sb = vecs.tile([128, 4], dt.float32, name="sm4_sb")
            nc.sync.dma_start(sm4_sb[:], smalls4.ap())
            ident_sb = vecs.tile([128, 128], dt.float16, name="ident_sb")
            nc.sync.dma_start(ident_sb[:], ident.ap())
            negc0_sb = sm4_sb[:, 0:1]
            negclv_sb = sm4_sb[:, 1:2]
            cabA_sb = sm4_sb[:, 2:3]
            cabB_sb = sm4_sb[:, 3:4]
            wub_sb = [vecs.tile([128, F + 5], dt.bfloat16, name=f"wub{k}")
                      for k in range(KC)]
            for k in range(KC):
                ks = slice(128 * k, 128 * (k + 1))
                nc.sync.dma_start(wub_sb[k][:], wub.ap()[ks, :])
            # hstrT / hstrT_my arrive pre-exponentiated from the host
            my_str = [stp.tile([128, S], dt.bfloat16, name=f"mystr{k}",
                               tag=f"mys{k}", bufs=1) for k in range(KC)]
            my_ctx = [stp.tile([128, S], dt.bfloat16, name=f"myctx{k}",
                               tag=f"myc{k}", bufs=1) for k in range(KC)]
            for k in range(KC):
                ks = slice(128 * k, 128 * (k + 1))
                nc.sync.dma_start(my_str[k][:], hstrT_my.ap()[ks, :])
                nc.sync.dma_start(my_ctx[k][:], hctxT_my.ap()[ks, :])

            # ---- src rows for my i-slice ----
            sigrow = work.tile([1, S], dt.float32, name="sigrow",
                               tag="rows", bufs=3)
            srcArow = work.tile([1, S], dt.float32, name="srcArow",
                                tag="rows", bufs=3)
            srcBrow = work.tile([1, S], dt.float32, name="srcBrow",
                                tag="rows", bufs=3)
            with tc.tile_pool(name="psrow", bufs=1, space="PSUM") as psrow:
                psr0 = psrow.tile([1, S], dt.float32, name="psr0")
                psr1 = psrow.tile([1, S], dt.float32, name="psr1")
                psra = psrow.tile([1, S], dt.float32, name="psra")
                for k in range(KC):
                    st, sp = (k == 0), (k == KC - 1)
                    for h in range(S // 512):
                        hs_ = slice(512 * h, 512 * (h + 1))
                        nc.tensor.matmul(psr0[:, hs_], wub_sb[k][:, F + 1:F + 2],
                                         my_str[k][:, hs_], start=st, stop=sp)
                        nc.tensor.matmul(psr1[:, hs_], wub_sb[k][:, F + 3:F + 4],
                                         my_str[k][:, hs_], start=st, stop=sp)
                        nc.tensor.matmul(psra[:, hs_], wub_sb[k][:, F + 4:F + 5],
                                         my_ctx[k][:, hs_], start=st, stop=sp)
                nc.vector.reciprocal(sigrow[:], psr0[:])
                nc.vector.tensor_copy(srcArow[:], psra[:])
                nc.vector.tensor_tensor(srcBrow[:], psr1[:], sigrow[:], OP.mult)

            ones_row = vecs.tile([1, 128], dt.float32, name="ones_row")
            nc.vector.memset(ones_row[:], 1.0)

            # broadcast rows -> [128, S] fp16 tiles with constants folded in
            bcA = vecs.tile([128, S], dt.float16, name="bcA")
            bcB = vecs.tile([128, S], dt.float16, name="bcB")
            with tc.tile_pool(name="ps0c", bufs=1, space="PSUM") as ps0c:
                psbc = ps0c.tile([128, S], dt.float32, name="psbc")
                psbc2 = ps0c.tile([128, S], dt.float32, name="psbc2")
                for h in range(S // 512):
                    hs_ = slice(512 * h, 512 * (h + 1))
                    nc.tensor.matmul(psbc[:, hs_], ones_row[:], srcArow[:, hs_],
                                     start=True, stop=True)
                    nc.tensor.matmul(psbc2[:, hs_], ones_row[:], srcBrow[:, hs_],
                                     start=True, stop=True)
                nc.vector.tensor_scalar(bcA[:], psbc[:], cabA_sb, None, OP.add)
                nc.vector.tensor_scalar(bcB[:], psbc2[:], cabB_sb, None, OP.add)

            # ---- phase 0/1 software-pipelined ----
            # per-group grid tiles (unique names: no cross-phase WAR hazards)
            NG = NCH // G0
            agrid_t = [vecs.tile([128, G0], dt.float32, name=f"ag{g}")
                       for g in range(NG)]
            bgrid_t = [vecs.tile([128, G0], dt.float32, name=f"bg{g}")
                       for g in range(NG)]
            explv_t = [vecs.tile([128, G0], dt.float32, name=f"lv{g}")
                       for g in range(NG)]
            explvb_t = [vecs.tile([128, G0], dt.bfloat16, name=f"lvb{g}")
                        for g in range(NG)]
            hc_sb = [hcpool.tile([128, F], dt.bfloat16, name=f"hc{c}")
                     for c in range(NCH)]
            slabs = []
            AST = 171  # psA chunk stride (fp32 elems); no PSUM bank crossings

            # per-slab mask layout (parity split): even slab 1 u16 + 3 u8
            # chunks, odd slab 2 + 2.

            with contextlib.ExitStack() as pctx:
                ps1 = pctx.enter_context(
                    tc.tile_pool(name="ps1", bufs=1, space="PSUM"))
                ps0 = pctx.enter_context(
                    tc.tile_pool(name="ps0", bufs=1, space="PSUM"))
                outT_ps = ps1.tile([F, S], dt.float32, name="outT_ps")
                rs_ps = ps1.tile([1, S], dt.float32, name="rs_ps")
                # reuses a psAg slot: only live after the last psAg is freed
                rbc_ps = ps0.tile([128, S // 2], dt.float32, name="rbc_ps",
                                  tag="psAg")

                def emit_slab_dma(u):
                    # one DMA per slab PAIR (even slab: 2 u16 + 2 u8 chunks,
                    # odd slab: 1 u16 + 3 u8)
                    slab_d = slabp.tile([128, 3 * S], dt.uint16,
                                        name="slabd", tag="slabd", bufs=2)
                    nc.sync.dma_start(
                        slab_d[:],
                        maskPd.ap()[:, 3 * u * S:3 * (u + 1) * S])
                    slab_g = slabp.tile([128, 5 * S], dt.uint8,
                                        name="slabg", tag="slabg", bufs=2)
                    nc.sync.dma_start(
                        slab_g[:],
                        maskPg.ap()[:, 5 * u * S:5 * (u + 1) * S])
                    slabs.append((slab_d[:, 0:2 * S], slab_g[:, 0:2 * S]))
                    slabs.append((slab_d[:, 2 * S:3 * S],
                                  slab_g[:, 2 * S:5 * S]))

                SGW = 2 * W0   # stream super-group: 2 phase-0 groups per DMA
                sgt = {}

                def emit_sg_dma(sg):
                    gs = slice(SGW * sg, SGW * (sg + 1))
                    hst = [stp.tile([128, SGW], dt.bfloat16, name=f"hstg{k}",
                                    tag=f"hst{k}", bufs=2) for k in range(KC)]
                    hct = [stp.tile([128, SGW], dt.bfloat16, name=f"hctg{k}",
                                    tag=f"hct{k}", bufs=2) for k in range(KC)]
                    for k in range(KC):
                        ks = slice(128 * k, 128 * (k + 1))
                        nc.sync.dma_start(hst[k][:], hstrT.ap()[ks, gs])
                        nc.sync.dma_start(hct[k][:], hctxT.ap()[ks, gs])
                    sgt[sg] = (hst, hct)

                def emit_p0_mm(g):
                    hstS, hctS = sgt[g // 2]
                    go = (g % 2) * W0
                    # one matmul per stationary load (129/2-col movings);
                    # dstA rides as column F of the psA block
                    psAg = ps0.tile([128, AST * G0 + 2 * G0], dt.float32,
                                    name="psAg")
                    SBO = AST * G0
                    # NOTE: keep the psA and psSB accumulation groups in
                    # separate time ranges — interleaving two open matmul
                    # accumulations in one PSUM bank corrupts the results.
                    for cc in range(G0):
                        cs = slice(go + 128 * cc, go + 128 * (cc + 1))
                        for k in range(KC):
                            st, sp = (k == 0), (k == KC - 1)
                            nc.tensor.matmul(
                                psAg[:, SBO + 2 * cc:SBO + 2 * cc + 2],
                                hstS[k][:, cs],
                                wub_sb[k][:, F + 1:F + 3], start=st, stop=sp)
                    for cc in range(G0):
                        cs = slice(go + 128 * cc, go + 128 * (cc + 1))
                        for k in range(KC):
                            st, sp = (k == 0), (k == KC - 1)
                            nc.tensor.matmul(
                                psAg[:, AST * cc:AST * cc + F + 1],
                                hctS[k][:, cs], wub_sb[k][:, 0:F + 1],
                                start=st, stop=sp)
                    return psAg

                def emit_p0_grid(g, psAg):
                    # batched grid math for this group of 8 chunks, then the
                    # hc' = hc * explv[j] copies (ACT, before the slab exp in
                    # the ACT queue so the psAg buffer frees early).
                    SBO = AST * G0
                    sg = work.tile([128, G0], dt.float32, name="sg",
                                   tag="sg", bufs=4)
                    nc.vector.reciprocal(sg[:],
                                         psAg[:, SBO:SBO + 2 * G0:2])
                    nc.vector.tensor_tensor(bgrid_t[g][:],
                                            psAg[:, SBO + 1:SBO + 2 * G0:2],
                                            sg[:], OP.mult)
                    nc.vector.tensor_copy(agrid_t[g][:],
                                          psAg[:, F:AST * G0:AST])
                    lvt = work.tile([128, G0], dt.float32, name="lvt",
                                    tag="sg", bufs=4)
                    nc.vector.tensor_tensor(lvt[:], agrid_t[g][:],
                                            bgrid_t[g][:], OP.add)
                    nc.scalar.activation(explv_t[g][:], lvt[:], AF.Exp,
                                         bias=negclv_sb, scale=0.01)
                    nc.scalar.copy(explvb_t[g][:], explv_t[g][:])
                    for cc in range(G0):
                        c = G0 * g + cc
                        nc.scalar.mul(hc_sb[c][:],
                                      psAg[:, AST * cc:AST * cc + F],
                                      explv_t[g][:, cc:cc + 1])

                def emit_tsp_pair(t, pair):
                    tAt = work.tile([128, 2 * S], dt.float16,
                                    name=f"tA{pair}", tag=f"tA{pair}",
                                    bufs=3)
                    tBt = work.tile([128, 2 * S], dt.float16,
                                    name=f"tB{pair}", tag=f"tB{pair}",
                                    bufs=3)
                    for h in range(2):
                        cc = 2 * pair + h
                        c = t * GRP + cc
                        gg, col = c // G0, c % G0
                        hs_ = slice(h * S, (h + 1) * S)
                        nc.vector.tensor_scalar(
                            tAt[:, hs_], bcA[:],
                            agrid_t[gg][:, col:col + 1],
                            0.0, OP.add, OP.max)
                        nc.vector.tensor_scalar(
                            tBt[:, hs_], bcB[:],
                            bgrid_t[gg][:, col:col + 1],
                            0.0, OP.add, OP.max)
                    return tAt, tBt

                def emit_z013(t):
                    # chunks {0,1}: DVE pair TT; chunk {3}: DVE single TT.
                    # zgrp layout: [0:2S] = chunks 0,1; [2S:3S] = chunk 3.
                    zgrp = grpp.tile([128, 3 * S], dt.float16, name="zgrp",
                                     bufs=3)
                    tAt0, tBt0 = emit_tsp_pair(t, 0)
                    nc.vector.tensor_tensor(zgrp[:, 0:2 * S], tAt0[:],
                                            tBt0[:], OP.add)
                    return zgrp

                def emit_z3_tt(t, zgrp, tAt1, tBt1):
                    nc.vector.tensor_tensor(zgrp[:, 2 * S:3 * S],
                                            tAt1[:, S:2 * S],
                                            tBt1[:, S:2 * S], OP.add)

                def emit_zmm_c2(t, tAt1, tBt1):
                    # chunk {2}: z = I @ tA + I @ tB on the tensor engine
                    # (PSUM fp32); read next cycle by its exp.
                    zps = ps0.tile([128, S], dt.float32, name="zps",
                                   tag="zps")
                    for q in range(2):
                        qs = slice(512 * q, 512 * (q + 1))
                        nc.tensor.matmul(zps[:, qs], ident_sb[:],
                                         tAt1[:, qs], start=True, stop=False)
                        nc.tensor.matmul(zps[:, qs], ident_sb[:],
                                         tBt1[:, qs], start=False, stop=True)
                    return zps

                def emit_exps(t, zgrp, zps):
                    # one cycle behind the z stage
                    Pgrp = grpp.tile([128, GRP * S], dt.bfloat16,
                                     name="Pgrp", bufs=3)
                    nc.scalar.activation(Pgrp[:, 2 * S:3 * S], zps[:],
                                         AF.Exp, bias=negc0_sb, scale=0.99)
                    nc.scalar.activation(Pgrp[:, 0:2 * S], zgrp[:, 0:2 * S],
                                         AF.Exp, bias=negc0_sb, scale=0.99)
                    nc.scalar.activation(Pgrp[:, 3 * S:4 * S],
                                         zgrp[:, 2 * S:3 * S],
                                         AF.Exp, bias=negc0_sb, scale=0.99)
                    return Pgrp

                def nd_of(t):
                    return 2 if t % 2 == 0 else 1

                def emit_mask_d(t, Pgrp):
                    # first nd chunks: batched uint16 mask TT on DVE (2x)
                    nd = nd_of(t)
                    slab_d, _ = slabs[t]
                    Pmd = pmp.tile([128, nd * S], dt.bfloat16,
                                   name="Pmd", tag="pmd", bufs=2)
                    nc.vector.tensor_tensor(Pmd[:], Pgrp[:, 0:nd * S],
                                            slab_d, OP.mult)
                    return Pmd

                def emit_mask_g(t, Pgrp):
                    # remaining chunks: batched uint8 mask TT on gpsimd
                    nd = nd_of(t)
                    _, slab_g = slabs[t]
                    Pmg = pmp.tile([128, (GRP - nd) * S], dt.bfloat16,
                                   name="Pmg", tag="pmg", bufs=2)
                    nc.gpsimd.tensor_tensor(Pmg[:], Pgrp[:, nd * S:GRP * S],
                                            slab_g, OP.mult)
                    return Pmg

                def emit_mm(t, src, ccs):
                    nd = nd_of(t)
                    for cc in ccs:
                        c = t * GRP + cc
                        gg, col = c // G0, c % G0
                        o = cc * S if cc < nd else (cc - nd) * S
                        st = (c == 0)
                        sp = (c == NCH - 1)
                        for h in range(S // 512):
                            hs_ = slice(512 * h, 512 * (h + 1))
                            so = slice(o + 512 * h, o + 512 * (h + 1))
                            nc.tensor.matmul(outT_ps[:, hs_], hc_sb[c][:],
                                             src[:, so], start=st, stop=sp)
                            nc.tensor.matmul(
                                rs_ps[:, hs_],
                                explvb_t[gg][:, col:col + 1],
                                src[:, so], start=st, stop=sp)

                # ---- slab-pipelined, PE kept saturated ----
                # per iter: DVE [maskd(t-1), TSP p1(t), TSP p0(t), zTT(t),
                # grid(g)]; PE [mmd(t-1), zmm2(t), zmm3(t), psA(g),
                # mmg(t-1)]; ACT [expc2(t), exp01(t), expc3(t), explv/hc(g)];
                # gp [maskg(t-1)].
                # Deep software pipeline at slab cadence: z-stage for slab t,
                # exps for t-1, mask+mm for t-2 — every cross-engine
                # dependency is at least one full cycle old, so each engine
                # streams its per-cycle menu without intra-cycle round trips.
                emit_sg_dma(0)
                psA0 = emit_p0_mm(0)
                psA1 = emit_p0_mm(1)
                emit_slab_dma(0)
                emit_sg_dma(1)
                emit_p0_grid(0, psA0)
                emit_p0_grid(1, psA1)
                Zs = {}
                Pg = {}
                NT = NSLAB

                psAs = {}

                def cycle(t):
                    # stages this cycle: mask+mm for slab t-2, grid/hc for
                    # group t+1 (its psA ran LAST cycle, so the grid ops
                    # never park at a queue head), exps for slab t-1, z for
                    # slab t, psA matmuls for group t+2.
                    if t % 2 == 0 and (t + 4) // 2 < NG // 2:
                        emit_sg_dma((t + 4) // 2)
                    if t % 2 == 0 and 0 < (t + 2) // 2 < NT // 2:
                        emit_slab_dma((t + 2) // 2)
                    tm = t - 2
                    Pmg = None
                    if 0 <= tm < NT:
                        Pmd = emit_mask_d(tm, Pg[tm])
                        emit_mm(tm, Pmd, tuple(range(nd_of(tm))))
                        Pmg = emit_mask_g(tm, Pg.pop(tm))
                    gg = t + 1
                    if 2 <= gg < NG:
                        emit_p0_grid(gg, psAs.pop(gg))
                    te = t - 1
                    if 0 <= te < NT:
                        zg_p, zps_p = Zs.pop(te)
                        Pg[te] = emit_exps(te, zg_p, zps_p)
                    if t < NT:
                        tAt1, tBt1 = emit_tsp_pair(t, 1)
                        zps = emit_zmm_c2(t, tAt1, tBt1)
                        zgrp = emit_z013(t)
                        emit_z3_tt(t, zgrp, tAt1, tBt1)
                        Zs[t] = (zgrp, zps)
                    gm = t + 2
                    if t < NT and gm < NG:
                        psAs[gm] = emit_p0_mm(gm)
                    if Pmg is not None:
                        emit_mm(tm, Pmg, tuple(range(nd_of(tm), GRP)))

                for t in range(NT + 2):
                    cycle(t)

                # normalize and write out
                rs_sb = work.tile([1, S], dt.float32, name="rs_sb",
                                  tag="rows", bufs=3)
                nc.vector.tensor_scalar_add(rs_sb[:], rs_ps[:], 1e-30)
                rrec = work.tile([1, S], dt.float32, name="rrec",
                                 tag="rows", bufs=3)
                nc.vector.reciprocal(rrec[:], rs_sb[:])
                rbc = work.tile([128, S], dt.float32, name="rbcs",
                                tag="big", bufs=2)
                for h in range(S // 512):
                    hs_ = slice(512 * h, 512 * (h + 1))
                    nc.tensor.matmul(rbc_ps[:, 0:512], ones_row[:],
                                     rrec[:, hs_], start=True, stop=True)
                    nc.vector.tensor_copy(rbc[:, hs_], rbc_ps[:, 0:512])
                out_sb = work.tile([F, S], dt.float32, name="out_sb",
                                   tag="big", bufs=2)
                nc.vector.tensor_tensor(out_sb[:], outT_ps[:], rbc[:], OP.mult)
                nc.sync.dma_start(outT.ap(), out_sb[:])

    nc.compile()
    return nc


def kernel(h_context, h_structure, edge_index, Wc_w, Wc_b, Ws_w, Ws_b,
           ac_w, as_w, Ws_coff, Wc_coff):
    from concourse.bass_utils import run_bass_kernel_spmd

    h_context = np.asarray(h_context, np.float32)
    h_structure = np.asarray(h_structure, np.float32)
    Wc_w = np.asarray(Wc_w, np.float32)
    Wc_b = np.asarray(Wc_b, np.float32)
    Ws_w = np.asarray(Ws_w, np.float32)
    Ws_b = np.asarray(Ws_b, np.float32)
    ac_w = np.asarray(ac_w, np.float32)
    as_w = np.asarray(as_w, np.float32)
    ei = np.asarray(edge_index)

    wA = float(abs(np.float32(np.asarray(Ws_coff)[0, 0])))  # scales alpha_c
    wB = float(abs(np.float32(np.asarray(Wc_coff)[0, 0])))  # scales alpha_s

    pA1 = wA * (Wc_w.T @ ac_w[0, :F])
    pA2 = wA * (Wc_w.T @ ac_w[0, F:])
    cA1 = wA * float(Wc_b @ ac_w[0, :F])
    cA2 = wA * float(Wc_b @ ac_w[0, F:])
    pB1 = wB * (Ws_w.T @ as_w[0, :F])
    pB2 = wB * (Ws_w.T @ as_w[0, F:])
    cB1 = wB * float(Ws_b @ as_w[0, :F])
    cB2 = wB * float(Ws_b @ as_w[0, F:])

    if 0 not in _BUILD_CACHE:
        _BUILD_CACHE[0] = _build_program()
    nc = _BUILD_CACHE[0]

    # adjacency, transposed + partition-major re-layout (edge -> 1)
    adjT = np.zeros((N, N), np.uint8)
    adjT[ei[1], ei[0]] = 1

    import ml_dtypes
    hctxT = np.ascontiguousarray(h_context.T)
    hstrT = np.ascontiguousarray(h_structure.T)
    # device consumes exp(h_structure) only: exponentiate on host
    hstrTe = np.exp(hstrT)
    hctxT16 = np.ascontiguousarray(hctxT.astype(ml_dtypes.bfloat16))
    hstrT16 = np.ascontiguousarray(hstrTe.astype(ml_dtypes.bfloat16))
    # wub = [wpack (F+1) | ones,pB2,pB1 (3) | pA1 (1)]
    wub_np = np.ascontiguousarray(np.concatenate(
        [Wc_w.T, pA2[:, None], np.ones((K, 1), np.float32),
         pB2[:, None], pB1[:, None], pA1[:, None]],
        axis=1).astype(np.float32))

    # host replicas of the projections for per-core range bounds (numerical
    # shim only; the bound cancels in the softmax normalization)
    srcA = h_context @ pA1 + (cA1 + cA2)
    dstA = h_context @ pA2
    e_str = np.exp(h_structure - h_structure.max(axis=1, keepdims=True))
    sm = e_str / e_str.sum(axis=1, keepdims=True)
    srcB = sm @ pB1 + (cB1 + cB2)
    dstB = sm @ pB2
    lv_full = 0.01 * (dstA + dstB + cA2 + cB2)
    Clv = float(lv_full.max())

    dA_max = float(dstA.max())
    dB_max = float(dstB.max())

    in_maps = []
    for d in range(NC):
        sl = slice(S * d, S * (d + 1))
        mA = max(0.0, float(srcA[sl].max()) + dA_max)
        mB = max(0.0, float(srcB[sl].max()) + dB_max)
        c0 = 0.99 * (mA + mB)
        mp = adjT[:, sl].reshape(N // 128, 128, S).transpose(1, 0, 2)
        # alternating split: even slab d={0,1}/g={2,3}; odd d={0}/g={1,2,3}
        d_idx, g_idx = [], []
        for t in range(NSLAB):
            nd = 2 if t % 2 == 0 else 1
            d_idx += [4 * t + j for j in range(nd)]
            g_idx += [4 * t + j for j in range(nd, 4)]
        maskPd = np.ascontiguousarray(
            mp[:, d_idx, :].reshape(128, len(d_idx) * S).astype(np.uint16))
        maskPg = np.ascontiguousarray(
            mp[:, g_idx, :].reshape(128, len(g_idx) * S))
        in_maps.append({
            "hctxT": hctxT16,
            "hstrT": hstrT16,
            "hctxT_my": np.ascontiguousarray(
                hctxT[:, sl].astype(ml_dtypes.bfloat16)),
            "hstrT_my": np.ascontiguousarray(
                hstrTe[:, sl].astype(ml_dtypes.bfloat16)),
            "wub": wub_np.astype(ml_dtypes.bfloat16),
            "maskPd": maskPd,
            "maskPg": maskPg,
            "smalls4": np.ascontiguousarray(np.broadcast_to(
                np.array([-c0, 0.01 * (cA2 + cB2) - Clv,
                          cA1 + cA2, cB1 + cB2], np.float32),
                (128, 4))),
            "ident": np.eye(128, dtype=np.float16),
        })

    res = run_bass_kernel_spmd(nc, in_maps, core_ids=list(range(NC)))
    out = np.empty((N, F), np.float32)
    for d in range(NC):
        out[S * d:S * (d + 1), :] = res.results[d]["outT"].T

    # hc bias: attention rows sum to 1, so + Wc_b exactly
    if np.any(Wc_b != 0.0):
        out += Wc_b[None, :]

    # rows with no edges: reference gives uniform attention = mean of hc
    row_deg = np.zeros(N, np.int64)
    np.add.at(row_deg, ei[0], 1)
    empty = row_deg == 0
    if empty.any():
        hc_host = h_context @ Wc_w.T + Wc_b
        out[empty, :] = hc_host.mean(axis=0)

    return out



# revision 50
# speedup vs baseline: 1.2927x; 1.2927x over previous
"""Trainium2 Bass kernel for nn_MixAttention (GAT-style mixed attention).

Strategy (8 cores, i-sharded over query rows, transposed compute):
  - Device computes scores in transposed layout [j on partitions, i free] so
    out^T += hc_chunk.T @ P^T_chunk contracts over partitions, no transposes.
  - lrelu decomposition: lrelu(x) = 0.01x + 0.99*relu(x). For both score
    terms the relu part is computed per entry; the linear part is rank-1:
    the per-i piece cancels in the row softmax, the per-j piece rides along
    as a multiplicative exp(lv[j]) folded into the mask op's scalar slot.
  - Per chunk: tA = relu(bcA + agrid_c) and tB = relu(bcB + bgrid_c) via
    single tensor_scalar(add, max-0) ops (fp16 packed = DVE 4x mode),
    z = tA + tB (fp16 TT, 2x), exp over half-groups of 2 chunks on ACT
    (scale=0.99, per-core bias tensor), then Pm = P * slab (plain mult;
    slab is the uint16 adjacency so masked entries are exactly 0; uint16
    keeps the DVE 2-byte fast path). exp(lv) is folded into the matmul
    stationaries instead: hc' = hc * explv (ACT copy with per-partition
    scale) and the rowsum stationary is an explv bf16 column.
  - PE accumulates out^T += hc'_c @ Pm and rowsum += explv_c @ Pm in bf16.
  - Phase-0 grids (dstA, dstB, sigma, lv) come from per-chunk PE matmuls
    into one PSUM block (129-wide psA with dstA as column F, 2-wide
    sigma/dstB pairs packed into the same banks) post-processed in batched
    [128, 8] ops. psA and sigma/dstB accumulation groups are emitted in
    disjoint time ranges: interleaving two open matmul accumulations in
    one PSUM bank corrupts results, as does issuing two matmuls on
    alternating stationaries with 1-column movings.
  - Phase 0 and phase 1 are software-pipelined: the emission loop issues
    phase-0 group g, then phase-1 groups 2(g-1) and 2(g-1)+1, then the hc
    copies of g, so every engine's in-order queue interleaves both phases.
    Grid tiles are per-group (unique names) to avoid whole-tile WAR
    hazards that would serialize the phases.
  - h inputs are bf16 (host-converted): per-query projection errors cancel
    in the row softmax; per-key errors are ~0.4% and measured end-to-end
    rel err is 4.8e-3 against the fp32 reference.
  - All per-core constants enter via input tensors (negc0/negclv/cabA/
    cabB), so a single compiled program serves every core and input set.
"""

import numpy as np

N = 8192
K = 256
F = 128
NC = 8
S = N // NC          # 1024 query rows per core
NCH = N // 128       # 64 j-chunks
KC = K // 128        # 2 contraction chunks
G0 = 8               # j-chunks per phase-0 stream group
W0 = G0 * 128
GRP = 4              # j-chunks per exp group / mask slab
NSLAB = NCH // GRP   # 16

_BUILD_CACHE = {}


def _build_program():
    import contextlib

    import concourse.bacc as bacc
    import concourse.tile as tile
    from concourse import mybir

    nc = bacc.Bacc("TRN2", target_bir_lowering=False, debug=False, num_devices=NC)
    dt = mybir.dt
    AF = mybir.ActivationFunctionType
    OP = mybir.AluOpType

    hctxT = nc.dram_tensor("hctxT", [K, N], dt.bfloat16, kind="ExternalInput")
    hstrT = nc.dram_tensor("hstrT", [K, N], dt.bfloat16, kind="ExternalInput")
    hctxT_my = nc.dram_tensor("hctxT_my", [K, S], dt.bfloat16,
                              kind="ExternalInput")
    hstrT_my = nc.dram_tensor("hstrT_my", [K, S], dt.bfloat16,
                              kind="ExternalInput")
    uB16 = nc.dram_tensor("uB16", [K, 3], dt.bfloat16, kind="ExternalInput")
    vA116 = nc.dram_tensor("vA116", [K, 1], dt.bfloat16, kind="ExternalInput")
    wpack = nc.dram_tensor("wpack", [K, F + 1], dt.bfloat16, kind="ExternalInput")
    maskPd = nc.dram_tensor("maskPd", [128, 2 * NSLAB * S], dt.uint16,
                            kind="ExternalInput")
    maskPg = nc.dram_tensor("maskPg", [128, 2 * NSLAB * S], dt.uint8,
                            kind="ExternalInput")
    negc0 = nc.dram_tensor("negc0", [128, 1], dt.float32, kind="ExternalInput")
    negclv = nc.dram_tensor("negclv", [128, 1], dt.float32, kind="ExternalInput")
    cabA = nc.dram_tensor("cabA", [128, 1], dt.float32, kind="ExternalInput")
    cabB = nc.dram_tensor("cabB", [128, 1], dt.float32, kind="ExternalInput")
    outT = nc.dram_tensor("outT", [F, S], dt.float32, kind="ExternalOutput")

    with tile.TileContext(nc) as tc:
        with contextlib.ExitStack() as ctx:
            vecs = ctx.enter_context(tc.tile_pool(name="vecs", bufs=1))
            hcpool = ctx.enter_context(tc.tile_pool(name="hc", bufs=1))
            stp = ctx.enter_context(tc.tile_pool(name="stream", bufs=2))
            work = ctx.enter_context(tc.tile_pool(name="work", bufs=3))
            grpp = ctx.enter_context(tc.tile_pool(name="grp", bufs=3))
            pmp = ctx.enter_context(tc.tile_pool(name="pm", bufs=6))
            slabp = ctx.enter_context(tc.tile_pool(name="slabp", bufs=3))

            # ---- small inputs ----
            wpack_sb = [vecs.tile([128, F + 1], dt.bfloat16, name=f"wp{k}")
                        for k in range(KC)]
            negc0_sb = vecs.tile([128, 1], dt.float32, name="negc0_sb")
            negclv_sb = vecs.tile([128, 1], dt.float32, name="negclv_sb")
            cabA_sb = vecs.tile([128, 1], dt.float32, name="cabA_sb")
            cabB_sb = vecs.tile([128, 1], dt.float32, name="cabB_sb")
            nc.sync.dma_start(negc0_sb[:], negc0.ap())
            nc.sync.dma_start(negclv_sb[:], negclv.ap())
            nc.sync.dma_start(cabA_sb[:], cabA.ap())
            nc.sync.dma_start(cabB_sb[:], cabB.ap())
            my_str = [stp.tile([128, S], dt.bfloat16, name=f"mystr{k}",
                               tag=f"hst{k}", bufs=3) for k in range(KC)]
            my_ctx = [stp.tile([128, S], dt.bfloat16, name=f"myctx{k}",
                               tag=f"hct{k}", bufs=3) for k in range(KC)]
            uB16_sb = [vecs.tile([128, 3], dt.bfloat16, name=f"uB16{k}")
                       for k in range(KC)]
            vA116_sb = [vecs.tile([128, 1], dt.bfloat16, name=f"vA116{k}")
                        for k in range(KC)]
            for k in range(KC):
                ks = slice(128 * k, 128 * (k + 1))
                nc.sync.dma_start(wpack_sb[k][:], wpack.ap()[ks, :])
                nc.sync.dma_start(my_str[k][:], hstrT_my.ap()[ks, :])
                nc.sync.dma_start(my_ctx[k][:], hctxT_my.ap()[ks, :])
                nc.sync.dma_start(uB16_sb[k][:], uB16.ap()[ks, :])
                nc.sync.dma_start(vA116_sb[k][:], vA116.ap()[ks, :])

            # ---- src rows for my i-slice ----
            sigrow = work.tile([1, S], dt.float32, name="sigrow", tag="u")
            srcArow = work.tile([1, S], dt.float32, name="srcArow", tag="tB")
            srcBrow = work.tile([1, S], dt.float32, name="srcBrow", tag="tA")
            with tc.tile_pool(name="psrow", bufs=1, space="PSUM") as psrow:
                psr0 = psrow.tile([1, S], dt.float32, name="psr0")
                psr1 = psrow.tile([1, S], dt.float32, name="psr1")
                psra = psrow.tile([1, S], dt.float32, name="psra")
                for k in range(KC):
                    st, sp = (k == 0), (k == KC - 1)
                    for h in range(S // 512):
                        hs_ = slice(512 * h, 512 * (h + 1))
                        nc.tensor.matmul(psr0[:, hs_], uB16_sb[k][:, 0:1],
                                         my_str[k][:, hs_], start=st, stop=sp)
                        nc.tensor.matmul(psr1[:, hs_], uB16_sb[k][:, 2:3],
                                         my_str[k][:, hs_], start=st, stop=sp)
                        nc.tensor.matmul(psra[:, hs_], vA116_sb[k][:, 0:1],
                                         my_ctx[k][:, hs_], start=st, stop=sp)
                nc.vector.reciprocal(sigrow[:], psr0[:])
                nc.vector.tensor_copy(srcArow[:], psra[:])
                nc.vector.tensor_tensor(srcBrow[:], psr1[:], sigrow[:], OP.mult)

            ones_row = vecs.tile([1, 128], dt.float32, name="ones_row")
            nc.vector.memset(ones_row[:], 1.0)

            # broadcast rows -> [128, S] fp16 tiles with constants folded in
            bcA = vecs.tile([128, S], dt.float16, name="bcA")
            bcB = vecs.tile([128, S], dt.float16, name="bcB")
            with tc.tile_pool(name="ps0c", bufs=1, space="PSUM") as ps0c:
                psbc = ps0c.tile([128, S], dt.float32, name="psbc")
                psbc2 = ps0c.tile([128, S], dt.float32, name="psbc2")
                for h in range(S // 512):
                    hs_ = slice(512 * h, 512 * (h + 1))
                    nc.tensor.matmul(psbc[:, hs_], ones_row[:], srcArow[:, hs_],
                                     start=True, stop=True)
                    nc.tensor.matmul(psbc2[:, hs_], ones_row[:], srcBrow[:, hs_],
                                     start=True, stop=True)
                nc.vector.tensor_scalar(bcA[:], psbc[:], cabA_sb[:], None, OP.add)
                nc.vector.tensor_scalar(bcB[:], psbc2[:], cabB_sb[:], None, OP.add)

            # ---- phase 0/1 software-pipelined ----
            # per-group grid tiles (unique names: no cross-phase WAR hazards)
            NG = NCH // G0
            agrid_t = [vecs.tile([128, G0], dt.float32, name=f"ag{g}")
                       for g in range(NG)]
            bgrid_t = [vecs.tile([128, G0], dt.float32, name=f"bg{g}")
                       for g in range(NG)]
            explv_t = [vecs.tile([128, G0], dt.float32, name=f"lv{g}")
                       for g in range(NG)]
            explvb_t = [vecs.tile([128, G0], dt.bfloat16, name=f"lvb{g}")
                        for g in range(NG)]
            hc_sb = [hcpool.tile([128, F], dt.bfloat16, name=f"hc{c}")
                     for c in range(NCH)]
            slabs = []
            AST = 171  # psA chunk stride (fp32 elems); no PSUM bank crossings

            with contextlib.ExitStack() as pctx:
                ps1 = pctx.enter_context(
                    tc.tile_pool(name="ps1", bufs=1, space="PSUM"))
                ps0 = pctx.enter_context(
                    tc.tile_pool(name="ps0", bufs=1, space="PSUM"))
                outT_ps = ps1.tile([F, S], dt.float32, name="outT_ps")
                rs_ps = ps1.tile([1, S], dt.float32, name="rs_ps")
                rbc_ps = ps1.tile([128, S // 2], dt.float32, name="rbc_ps")

                def emit_p0(g):
                    for t in (2 * g, 2 * g + 1):
                        slab_d = slabp.tile([128, 2 * S], dt.uint16,
                                            name="slabd", bufs=3)
                        nc.sync.dma_start(
                            slab_d[:],
                            maskPd.ap()[:, 2 * t * S:2 * (t + 1) * S])
                        slab_g = slabp.tile([128, 2 * S], dt.uint8,
                                            name="slabg", bufs=3)
                        nc.sync.dma_start(
                            slab_g[:],
                            maskPg.ap()[:, 2 * t * S:2 * (t + 1) * S])
                        slabs.append((slab_d, slab_g))
                    gs = slice(W0 * g, W0 * (g + 1))
                    hst = [stp.tile([128, W0], dt.bfloat16, name=f"hstg{k}",
                                    tag=f"hst{k}", bufs=3) for k in range(KC)]
                    hct = [stp.tile([128, W0], dt.bfloat16, name=f"hctg{k}",
                                    tag=f"hct{k}", bufs=3) for k in range(KC)]
                    for k in range(KC):
                        ks = slice(128 * k, 128 * (k + 1))
                        nc.sync.dma_start(hst[k][:], hstrT.ap()[ks, gs])
                        nc.sync.dma_start(hct[k][:], hctxT.ap()[ks, gs])
                    # one matmul per stationary load (129/2-col movings);
                    # dstA rides as column F of the psA block
                    psAg = ps0.tile([128, AST * G0 + 2 * G0], dt.float32,
                                    name="psAg")
                    SBO = AST * G0
                    # NOTE: keep the psA and psSB accumulation groups in
                    # separate time ranges — interleaving two open matmul
                    # accumulations in one PSUM bank corrupts the results.
                    for cc in range(G0):
                        cs = slice(128 * cc, 128 * (cc + 1))
                        for k in range(KC):
                            st, sp = (k == 0), (k == KC - 1)
                            nc.tensor.matmul(
                                psAg[:, SBO + 2 * cc:SBO + 2 * cc + 2],
                                hst[k][:, cs],
                                uB16_sb[k][:, 0:2], start=st, stop=sp)
                    for cc in range(G0):
                        cs = slice(128 * cc, 128 * (cc + 1))
                        for k in range(KC):
                            st, sp = (k == 0), (k == KC - 1)
                            nc.tensor.matmul(
                                psAg[:, AST * cc:AST * cc + F + 1],
                                hct[k][:, cs], wpack_sb[k][:, 0:F + 1],
                                start=st, stop=sp)
                    # batched grid math for this group of 8 chunks
                    sg = work.tile([128, G0], dt.float32, name="sg", tag="sg")
                    nc.vector.reciprocal(sg[:],
                                         psAg[:, SBO:SBO + 2 * G0:2])
                    nc.vector.tensor_tensor(bgrid_t[g][:],
                                            psAg[:, SBO + 1:SBO + 2 * G0:2],
                                            sg[:], OP.mult)
                    nc.vector.tensor_copy(agrid_t[g][:],
                                          psAg[:, F:AST * G0:AST])
                    lvt = work.tile([128, G0], dt.float32, name="lvt", tag="sg")
                    nc.vector.tensor_tensor(lvt[:], agrid_t[g][:],
                                            bgrid_t[g][:], OP.add)
                    nc.scalar.activation(explv_t[g][:], lvt[:], AF.Exp,
                                         bias=negclv_sb[:], scale=0.01)
                    nc.vector.tensor_copy(explvb_t[g][:], explv_t[g][:])

                    def hc_copies(g=g, psAg=psAg):
                        # hc' = hc * explv[j] (per-partition ACT-copy scale)
                        for cc in range(G0):
                            c = G0 * g + cc
                            nc.scalar.mul(hc_sb[c][:],
                                          psAg[:, AST * cc:AST * cc + F],
                                          explv_t[g][:, cc:cc + 1])
                    return hc_copies

                def emit_z(t):
                    zgrp = grpp.tile([128, GRP * S], dt.float16, name="zgrp")
                    Pgrp = grpp.tile([128, GRP * S], dt.bfloat16, name="Pgrp")
                    for pair in range(GRP // 2):
                        tAt = work.tile([128, 2 * S], dt.float16, name="tA",
                                        tag="tA")
                        tBt = work.tile([128, 2 * S], dt.float16, name="tB",
                                        tag="tB")
                        for h in range(2):
                            cc = 2 * pair + h
                            c = t * GRP + cc
                            gg, col = c // G0, c % G0
                            hs_ = slice(h * S, (h + 1) * S)
                            nc.vector.tensor_scalar(
                                tAt[:, hs_], bcA[:],
                                agrid_t[gg][:, col:col + 1],
                                0.0, OP.add, OP.max)
                            nc.vector.tensor_scalar(
                                tBt[:, hs_], bcB[:],
                                bgrid_t[gg][:, col:col + 1],
                                0.0, OP.add, OP.max)
                        o = pair * 2 * S
                        nc.vector.tensor_tensor(zgrp[:, o:o + 2 * S], tAt[:],
                                                tBt[:], OP.add)
                    nc.scalar.activation(Pgrp[:], zgrp[:], AF.Exp,
                                         bias=negc0_sb[:], scale=0.99)
                    return Pgrp

                def emit_mm(t, Pgrp):
                    slab_d, slab_g = slabs[t]
                    Pmd = pmp.tile([128, 2 * S], dt.bfloat16, name="Pmd",
                                   tag="pmd", bufs=3)
                    Pmg = pmp.tile([128, 2 * S], dt.bfloat16, name="Pmg",
                                   tag="pmg", bufs=3)
                    nc.vector.tensor_tensor(Pmd[:], Pgrp[:, 0:2 * S],
                                            slab_d[:], OP.mult)
                    nc.gpsimd.tensor_tensor(Pmg[:], Pgrp[:, 2 * S:4 * S],
                                            slab_g[:], OP.mult)
                    for cc in range(GRP):
                        c = t * GRP + cc
                        gg, col = c // G0, c % G0
                        src_ = Pmd if cc < 2 else Pmg
                        o = (cc % 2) * S
                        st = (c == 0)
                        sp = (c == NCH - 1)
                        for h in range(S // 512):
                            hs_ = slice(512 * h, 512 * (h + 1))
                            so = slice(o + 512 * h, o + 512 * (h + 1))
                            nc.tensor.matmul(outT_ps[:, hs_], hc_sb[c][:],
                                             src_[:, so], start=st, stop=sp)
                            nc.tensor.matmul(
                                rs_ps[:, hs_],
                                explvb_t[gg][:, col:col + 1],
                                src_[:, so], start=st, stop=sp)

                for g in range(NG):
                    hc_cp = emit_p0(g)
                    if g >= 1:
                        for tt in (2 * (g - 1), 2 * (g - 1) + 1):
                            emit_mm(tt, emit_z(tt))
                    hc_cp()
                for tt in (2 * NG - 2, 2 * NG - 1):
                    emit_mm(tt, emit_z(tt))

                # normalize and write out
                rs_sb = work.tile([1, S], dt.float32, name="rs_sb", tag="tB")
                nc.vector.tensor_scalar_add(rs_sb[:], rs_ps[:], 1e-30)
                rrec = work.tile([1, S], dt.float32, name="rrec", tag="sg")
                nc.vector.reciprocal(rrec[:], rs_sb[:])
                rbc = work.tile([128, S], dt.float32, name="rbcs", tag="u")
                for h in range(S // 512):
                    hs_ = slice(512 * h, 512 * (h + 1))
                    nc.tensor.matmul(rbc_ps[:, 0:512], ones_row[:],
                                     rrec[:, hs_], start=True, stop=True)
                    nc.vector.tensor_copy(rbc[:, hs_], rbc_ps[:, 0:512])
                out_sb = work.tile([F, S], dt.float32, name="out_sb", tag="tA")
                nc.vector.tensor_tensor(out_sb[:], outT_ps[:], rbc[:], OP.mult)
                nc.sync.dma_start(outT.ap(), out_sb[:])

    nc.compile()
    return nc


def kernel(h_context, h_structure, edge_index, Wc_w, Wc_b, Ws_w, Ws_b,
           ac_w, as_w, Ws_coff, Wc_coff):
    from concourse.bass_utils import run_bass_kernel_spmd

    h_context = np.asarray(h_context, np.float32)
    h_structure = np.asarray(h_structure, np.float32)
    Wc_w = np.asarray(Wc_w, np.float32)
    Wc_b = np.asarray(Wc_b, np.float32)
    Ws_w = np.asarray(Ws_w, np.float32)
    Ws_b = np.asarray(Ws_b, np.float32)
    ac_w = np.asarray(ac_w, np.float32)
    as_w = np.asarray(as_w, np.float32)
    ei = np.asarray(edge_index)

    wA = float(abs(np.float32(np.asarray(Ws_coff)[0, 0])))  # scales alpha_c
    wB = float(abs(np.float32(np.asarray(Wc_coff)[0, 0])))  # scales alpha_s

    pA1 = wA * (Wc_w.T @ ac_w[0, :F])
    pA2 = wA * (Wc_w.T @ ac_w[0, F:])
    cA1 = wA * float(Wc_b @ ac_w[0, :F])
    cA2 = wA * float(Wc_b @ ac_w[0, F:])
    pB1 = wB * (Ws_w.T @ as_w[0, :F])
    pB2 = wB * (Ws_w.T @ as_w[0, F:])
    cB1 = wB * float(Ws_b @ as_w[0, :F])
    cB2 = wB * float(Ws_b @ as_w[0, F:])

    if 0 not in _BUILD_CACHE:
        _BUILD_CACHE[0] = _build_program()
    nc = _BUILD_CACHE[0]

    # adjacency, transposed + partition-major re-layout (edge -> 1)
    adjT = np.zeros((N, N), np.uint8)
    adjT[ei[1], ei[0]] = 1

    import ml_dtypes
    hctxT = np.ascontiguousarray(h_context.T)
    hstrT = np.ascontiguousarray(h_structure.T)
    hstrTe = np.exp(hstrT)
    hctxT16 = np.ascontiguousarray(hctxT.astype(ml_dtypes.bfloat16))
    hstrT16 = np.ascontiguousarray(hstrTe.astype(ml_dtypes.bfloat16))
    wpack_np = np.ascontiguousarray(
        np.concatenate([Wc_w.T, pA2[:, None]], axis=1).astype(np.float32))
    uB_np = np.ascontiguousarray(np.stack(
        [np.ones(K, np.float32), pB2, pB1], axis=1).astype(np.float32))
    vA1_np = np.ascontiguousarray(pA1[:, None].astype(np.float32))

    # host replicas of the projections for per-core range bounds (numerical
    # shim only; the bound cancels in the softmax normalization)
    srcA = h_context @ pA1 + (cA1 + cA2)
    dstA = h_context @ pA2
    e_str = np.exp(h_structure - h_structure.max(axis=1, keepdims=True))
    sm = e_str / e_str.sum(axis=1, keepdims=True)
    srcB = sm @ pB1 + (cB1 + cB2)
    dstB = sm @ pB2
    lv_full = 0.01 * (dstA + dstB + cA2 + cB2)
    Clv = float(lv_full.max())

    dA_max = float(dstA.max())
    dB_max = float(dstB.max())

    in_maps = []
    for d in range(NC):
        sl = slice(S * d, S * (d + 1))
        mA = max(0.0, float(srcA[sl].max()) + dA_max)
        mB = max(0.0, float(srcB[sl].max()) + dB_max)
        c0 = 0.99 * (mA + mB)
        mp = adjT[:, sl].reshape(N // 128, 128, S).transpose(1, 0, 2)
        d_idx = [c for c in range(NCH) if c % 4 < 2]
        g_idx = [c for c in range(NCH) if c % 4 >= 2]
        maskPd = np.ascontiguousarray(
            mp[:, d_idx, :].reshape(128, 2 * NSLAB * S).astype(np.uint16))
        maskPg = np.ascontiguousarray(
            mp[:, g_idx, :].reshape(128, 2 * NSLAB * S))
        in_maps.append({
            "hctxT": hctxT16,
            "hstrT": hstrT16,
            "hctxT_my": np.ascontiguousarray(
                hctxT[:, sl].astype(ml_dtypes.bfloat16)),
            "hstrT_my": np.ascontiguousarray(
                hstrTe[:, sl].astype(ml_dtypes.bfloat16)),
            "uB16": uB_np.astype(ml_dtypes.bfloat16),
            "vA116": vA1_np.astype(ml_dtypes.bfloat16),
            "wpack": wpack_np.astype(ml_dtypes.bfloat16),
            "maskPd": maskPd,
            "maskPg": maskPg,
            "negc0": np.full((128, 1), -np.float32(c0), np.float32),
            "negclv": np.full((128, 1),
                              np.float32(0.01 * (cA2 + cB2) - Clv),
                              np.float32),
            "cabA": np.full((128, 1), np.float32(cA1 + cA2), np.float32),
            "cabB": np.full((128, 1), np.float32(cB1 + cB2), np.float32),
        })

    res = run_bass_kernel_spmd(nc, in_maps, core_ids=list(range(NC)))
    out = np.empty((N, F), np.float32)
    for d in range(NC):
        out[S * d:S * (d + 1), :] = res.results[d]["outT"].T

    # hc bias: attention rows sum to 1, so + Wc_b exactly
    if np.any(Wc_b != 0.0):
        out += Wc_b[None, :]

    # rows with no edges: reference gives uniform attention = mean of hc
    row_deg = np.zeros(N, np.int64)
    np.add.at(row_deg, ei[0], 1)
    empty = row_deg == 0
    if empty.any():
        hc_host = h_context @ Wc_w.T + Wc_b
        out[empty, :] = hc_host.mean(axis=0)

    return out



# revision 51
# speedup vs baseline: 1.3265x; 1.0262x over previous
"""Trainium2 Bass kernel for nn_MixAttention (GAT-style mixed attention).

Strategy (8 cores, i-sharded over query rows, transposed compute):
  - Device computes scores in transposed layout [j on partitions, i free] so
    out^T += hc_chunk.T @ P^T_chunk contracts over partitions, no transposes.
  - lrelu decomposition: lrelu(x) = 0.01x + 0.99*relu(x). For both score
    terms the relu part is computed per entry; the linear part is rank-1:
    the per-i piece cancels in the row softmax, the per-j piece rides along
    as a multiplicative exp(lv[j]) folded into the mask op's scalar slot.
  - Per chunk: tA = relu(bcA + agrid_c) and tB = relu(bcB + bgrid_c) via
    single tensor_scalar(add, max-0) ops (fp16 packed = DVE 4x mode),
    z = tA + tB (fp16 TT, 2x), exp over half-groups of 2 chunks on ACT
    (scale=0.99, per-core bias tensor), then Pm = P * slab (plain mult;
    slab is the uint16 adjacency so masked entries are exactly 0; uint16
    keeps the DVE 2-byte fast path). exp(lv) is folded into the matmul
    stationaries instead: hc' = hc * explv (ACT copy with per-partition
    scale) and the rowsum stationary is an explv bf16 column.
  - PE accumulates out^T += hc'_c @ Pm and rowsum += explv_c @ Pm in bf16.
  - Phase-0 grids (dstA, dstB, sigma, lv) come from per-chunk PE matmuls
    into one PSUM block (129-wide psA with dstA as column F, 2-wide
    sigma/dstB pairs packed into the same banks) post-processed in batched
    [128, 8] ops. psA and sigma/dstB accumulation groups are emitted in
    disjoint time ranges: interleaving two open matmul accumulations in
    one PSUM bank corrupts results, as does issuing two matmuls on
    alternating stationaries with 1-column movings.
  - Phase 0 and phase 1 are software-pipelined: the emission loop issues
    phase-0 group g, then phase-1 groups 2(g-1) and 2(g-1)+1, then the hc
    copies of g, so every engine's in-order queue interleaves both phases.
    Grid tiles are per-group (unique names) to avoid whole-tile WAR
    hazards that would serialize the phases.
  - h inputs are bf16 (host-converted): per-query projection errors cancel
    in the row softmax; per-key errors are ~0.4% and measured end-to-end
    rel err is 4.8e-3 against the fp32 reference.
  - All per-core constants enter via input tensors (negc0/negclv/cabA/
    cabB), so a single compiled program serves every core and input set.
"""

import numpy as np

N = 8192
K = 256
F = 128
NC = 8
S = N // NC          # 1024 query rows per core
NCH = N // 128       # 64 j-chunks
KC = K // 128        # 2 contraction chunks
G0 = 8               # j-chunks per phase-0 stream group
W0 = G0 * 128
GRP = 4              # j-chunks per exp group / mask slab
NSLAB = NCH // GRP   # 16

_BUILD_CACHE = {}


def _build_program():
    import contextlib

    import concourse.bacc as bacc
    import concourse.tile as tile
    from concourse import mybir

    nc = bacc.Bacc("TRN2", target_bir_lowering=False, debug=False, num_devices=NC)
    dt = mybir.dt
    AF = mybir.ActivationFunctionType
    OP = mybir.AluOpType

    hctxT = nc.dram_tensor("hctxT", [K, N], dt.bfloat16, kind="ExternalInput")
    hstrT = nc.dram_tensor("hstrT", [K, N], dt.bfloat16, kind="ExternalInput")
    hctxT_my = nc.dram_tensor("hctxT_my", [K, S], dt.bfloat16,
                              kind="ExternalInput")
    hstrT_my = nc.dram_tensor("hstrT_my", [K, S], dt.bfloat16,
                              kind="ExternalInput")
    # wub = [wpack (F+1) | ones,pB2,pB1 (3) | pA1 (1)]: one DMA per k-chunk
    wub = nc.dram_tensor("wub", [K, F + 5], dt.bfloat16, kind="ExternalInput")
    maskPd = nc.dram_tensor("maskPd", [128, 2 * NSLAB * S], dt.uint16,
                            kind="ExternalInput")
    maskPg = nc.dram_tensor("maskPg", [128, 2 * NSLAB * S], dt.uint8,
                            kind="ExternalInput")
    smalls4 = nc.dram_tensor("smalls4", [128, 4], dt.float32,
                             kind="ExternalInput")
    outT = nc.dram_tensor("outT", [F, S], dt.float32, kind="ExternalOutput")

    with tile.TileContext(nc) as tc:
        with contextlib.ExitStack() as ctx:
            vecs = ctx.enter_context(tc.tile_pool(name="vecs", bufs=1))
            hcpool = ctx.enter_context(tc.tile_pool(name="hc", bufs=1))
            stp = ctx.enter_context(tc.tile_pool(name="stream", bufs=2))
            work = ctx.enter_context(tc.tile_pool(name="work", bufs=3))
            grpp = ctx.enter_context(tc.tile_pool(name="grp", bufs=3))
            pmp = ctx.enter_context(tc.tile_pool(name="pm", bufs=6))
            slabp = ctx.enter_context(tc.tile_pool(name="slabp", bufs=3))

            # ---- small inputs (3 + 4 DMAs) ----
            sm4_sb = vecs.tile([128, 4], dt.float32, name="sm4_sb")
            nc.sync.dma_start(sm4_sb[:], smalls4.ap())
            negc0_sb = sm4_sb[:, 0:1]
            negclv_sb = sm4_sb[:, 1:2]
            cabA_sb = sm4_sb[:, 2:3]
            cabB_sb = sm4_sb[:, 3:4]
            wub_sb = [vecs.tile([128, F + 5], dt.bfloat16, name=f"wub{k}")
                      for k in range(KC)]
            my_str = [stp.tile([128, S], dt.bfloat16, name=f"mystr{k}",
                               tag=f"hst{k}", bufs=3) for k in range(KC)]
            my_ctx = [stp.tile([128, S], dt.bfloat16, name=f"myctx{k}",
                               tag=f"hct{k}", bufs=3) for k in range(KC)]
            for k in range(KC):
                ks = slice(128 * k, 128 * (k + 1))
                nc.sync.dma_start(wub_sb[k][:], wub.ap()[ks, :])
                nc.sync.dma_start(my_str[k][:], hstrT_my.ap()[ks, :])
                nc.sync.dma_start(my_ctx[k][:], hctxT_my.ap()[ks, :])

            # ---- src rows for my i-slice ----
            sigrow = work.tile([1, S], dt.float32, name="sigrow", tag="u")
            srcArow = work.tile([1, S], dt.float32, name="srcArow", tag="tB")
            srcBrow = work.tile([1, S], dt.float32, name="srcBrow", tag="tA")
            with tc.tile_pool(name="psrow", bufs=1, space="PSUM") as psrow:
                psr0 = psrow.tile([1, S], dt.float32, name="psr0")
                psr1 = psrow.tile([1, S], dt.float32, name="psr1")
                psra = psrow.tile([1, S], dt.float32, name="psra")
                for k in range(KC):
                    st, sp = (k == 0), (k == KC - 1)
                    for h in range(S // 512):
                        hs_ = slice(512 * h, 512 * (h + 1))
                        nc.tensor.matmul(psr0[:, hs_], wub_sb[k][:, F + 1:F + 2],
                                         my_str[k][:, hs_], start=st, stop=sp)
                        nc.tensor.matmul(psr1[:, hs_], wub_sb[k][:, F + 3:F + 4],
                                         my_str[k][:, hs_], start=st, stop=sp)
                        nc.tensor.matmul(psra[:, hs_], wub_sb[k][:, F + 4:F + 5],
                                         my_ctx[k][:, hs_], start=st, stop=sp)
                nc.vector.reciprocal(sigrow[:], psr0[:])
                nc.scalar.copy(srcArow[:], psra[:])
                nc.vector.tensor_tensor(srcBrow[:], psr1[:], sigrow[:], OP.mult)

            ones_row = vecs.tile([1, 128], dt.float32, name="ones_row")
            nc.vector.memset(ones_row[:], 1.0)

            # broadcast rows -> [128, S] fp16 tiles with constants folded in
            bcA = vecs.tile([128, S], dt.float16, name="bcA")
            bcB = vecs.tile([128, S], dt.float16, name="bcB")
            with tc.tile_pool(name="ps0c", bufs=1, space="PSUM") as ps0c:
                psbc = ps0c.tile([128, S], dt.float32, name="psbc")
                psbc2 = ps0c.tile([128, S], dt.float32, name="psbc2")
                for h in range(S // 512):
                    hs_ = slice(512 * h, 512 * (h + 1))
                    nc.tensor.matmul(psbc[:, hs_], ones_row[:], srcArow[:, hs_],
                                     start=True, stop=True)
                    nc.tensor.matmul(psbc2[:, hs_], ones_row[:], srcBrow[:, hs_],
                                     start=True, stop=True)
                nc.vector.tensor_scalar(bcA[:], psbc[:], cabA_sb, None, OP.add)
                nc.vector.tensor_scalar(bcB[:], psbc2[:], cabB_sb, None, OP.add)

            # ---- phase 0/1 software-pipelined ----
            # per-group grid tiles (unique names: no cross-phase WAR hazards)
            NG = NCH // G0
            agrid_t = [vecs.tile([128, G0], dt.float32, name=f"ag{g}")
                       for g in range(NG)]
            bgrid_t = [vecs.tile([128, G0], dt.float32, name=f"bg{g}")
                       for g in range(NG)]
            explv_t = [vecs.tile([128, G0], dt.float32, name=f"lv{g}")
                       for g in range(NG)]
            explvb_t = [vecs.tile([128, G0], dt.bfloat16, name=f"lvb{g}")
                        for g in range(NG)]
            hc_sb = [hcpool.tile([128, F], dt.bfloat16, name=f"hc{c}")
                     for c in range(NCH)]
            slabs = []
            AST = 171  # psA chunk stride (fp32 elems); no PSUM bank crossings

            with contextlib.ExitStack() as pctx:
                ps1 = pctx.enter_context(
                    tc.tile_pool(name="ps1", bufs=1, space="PSUM"))
                ps0 = pctx.enter_context(
                    tc.tile_pool(name="ps0", bufs=1, space="PSUM"))
                outT_ps = ps1.tile([F, S], dt.float32, name="outT_ps")
                rs_ps = ps1.tile([1, S], dt.float32, name="rs_ps")
                rbc_ps = ps1.tile([128, S // 2], dt.float32, name="rbc_ps")

                def emit_p0(g):
                    t0 = 2 * g
                    slab_d = slabp.tile([128, 4 * S], dt.uint16,
                                        name="slabd", bufs=2)
                    nc.sync.dma_start(
                        slab_d[:],
                        maskPd.ap()[:, 2 * t0 * S:2 * (t0 + 2) * S])
                    slab_g = slabp.tile([128, 4 * S], dt.uint8,
                                        name="slabg", bufs=2)
                    nc.sync.dma_start(
                        slab_g[:],
                        maskPg.ap()[:, 2 * t0 * S:2 * (t0 + 2) * S])
                    slabs.append((slab_d[:, 0:2 * S], slab_g[:, 0:2 * S]))
                    slabs.append((slab_d[:, 2 * S:4 * S],
                                  slab_g[:, 2 * S:4 * S]))
                    gs = slice(W0 * g, W0 * (g + 1))
                    hst = [stp.tile([128, W0], dt.bfloat16, name=f"hstg{k}",
                                    tag=f"hst{k}", bufs=3) for k in range(KC)]
                    hct = [stp.tile([128, W0], dt.bfloat16, name=f"hctg{k}",
                                    tag=f"hct{k}", bufs=3) for k in range(KC)]
                    for k in range(KC):
                        ks = slice(128 * k, 128 * (k + 1))
                        nc.sync.dma_start(hst[k][:], hstrT.ap()[ks, gs])
                        nc.sync.dma_start(hct[k][:], hctxT.ap()[ks, gs])
                    # one matmul per stationary load (129/2-col movings);
                    # dstA rides as column F of the psA block
                    psAg = ps0.tile([128, AST * G0 + 2 * G0], dt.float32,
                                    name="psAg")
                    SBO = AST * G0
                    # NOTE: keep the psA and psSB accumulation groups in
                    # separate time ranges — interleaving two open matmul
                    # accumulations in one PSUM bank corrupts the results.
                    for cc in range(G0):
                        cs = slice(128 * cc, 128 * (cc + 1))
                        for k in range(KC):
                            st, sp = (k == 0), (k == KC - 1)
                            nc.tensor.matmul(
                                psAg[:, SBO + 2 * cc:SBO + 2 * cc + 2],
                                hst[k][:, cs],
                                wub_sb[k][:, F + 1:F + 3], start=st, stop=sp)
                    for cc in range(G0):
                        cs = slice(128 * cc, 128 * (cc + 1))
                        for k in range(KC):
                            st, sp = (k == 0), (k == KC - 1)
                            nc.tensor.matmul(
                                psAg[:, AST * cc:AST * cc + F + 1],
                                hct[k][:, cs], wub_sb[k][:, 0:F + 1],
                                start=st, stop=sp)
                    # batched grid math for this group of 8 chunks
                    sg = work.tile([128, G0], dt.float32, name="sg", tag="sg")
                    nc.vector.reciprocal(sg[:],
                                         psAg[:, SBO:SBO + 2 * G0:2])
                    nc.vector.tensor_tensor(bgrid_t[g][:],
                                            psAg[:, SBO + 1:SBO + 2 * G0:2],
                                            sg[:], OP.mult)
                    nc.vector.tensor_copy(agrid_t[g][:],
                                          psAg[:, F:AST * G0:AST])
                    lvt = work.tile([128, G0], dt.float32, name="lvt", tag="sg")
                    nc.vector.tensor_tensor(lvt[:], agrid_t[g][:],
                                            bgrid_t[g][:], OP.add)
                    nc.scalar.activation(explv_t[g][:], lvt[:], AF.Exp,
                                         bias=negclv_sb, scale=0.01)
                    nc.scalar.copy(explvb_t[g][:], explv_t[g][:])

                    def hc_copies(g=g, psAg=psAg):
                        # hc' = hc * explv[j] (per-partition ACT-copy scale)
                        for cc in range(G0):
                            c = G0 * g + cc
                            nc.scalar.mul(hc_sb[c][:],
                                          psAg[:, AST * cc:AST * cc + F],
                                          explv_t[g][:, cc:cc + 1])
                    return hc_copies

                def emit_z(t):
                    zgrp = grpp.tile([128, GRP * S], dt.float16, name="zgrp")
                    Pgrp = grpp.tile([128, GRP * S], dt.bfloat16, name="Pgrp")
                    for pair in range(GRP // 2):
                        tAt = work.tile([128, 2 * S], dt.float16, name="tA",
                                        tag="tA")
                        tBt = work.tile([128, 2 * S], dt.float16, name="tB",
                                        tag="tB")
                        for h in range(2):
                            cc = 2 * pair + h
                            c = t * GRP + cc
                            gg, col = c // G0, c % G0
                            hs_ = slice(h * S, (h + 1) * S)
                            nc.vector.tensor_scalar(
                                tAt[:, hs_], bcA[:],
                                agrid_t[gg][:, col:col + 1],
                                0.0, OP.add, OP.max)
                            nc.vector.tensor_scalar(
                                tBt[:, hs_], bcB[:],
                                bgrid_t[gg][:, col:col + 1],
                                0.0, OP.add, OP.max)
                        o = pair * 2 * S
                        nc.vector.tensor_tensor(zgrp[:, o:o + 2 * S], tAt[:],
                                                tBt[:], OP.add)
                    nc.scalar.activation(Pgrp[:], zgrp[:], AF.Exp,
                                         bias=negc0_sb, scale=0.99)
                    return Pgrp

                def emit_mm(t, Pgrp):
                    slab_d, slab_g = slabs[t]
                    Pmd = pmp.tile([128, 2 * S], dt.bfloat16, name="Pmd",
                                   tag="pmd", bufs=3)
                    Pmg = pmp.tile([128, 2 * S], dt.bfloat16, name="Pmg",
                                   tag="pmg", bufs=3)
                    nc.vector.tensor_tensor(Pmd[:], Pgrp[:, 0:2 * S],
                                            slab_d, OP.mult)
                    nc.gpsimd.tensor_tensor(Pmg[:], Pgrp[:, 2 * S:4 * S],
                                            slab_g, OP.mult)
                    for cc in range(GRP):
                        c = t * GRP + cc
                        gg, col = c // G0, c % G0
                        src_ = Pmd if cc < 2 else Pmg
                        o = (cc % 2) * S
                        st = (c == 0)
                        sp = (c == NCH - 1)
                        for h in range(S // 512):
                            hs_ = slice(512 * h, 512 * (h + 1))
                            so = slice(o + 512 * h, o + 512 * (h + 1))
                            nc.tensor.matmul(outT_ps[:, hs_], hc_sb[c][:],
                                             src_[:, so], start=st, stop=sp)
                            nc.tensor.matmul(
                                rs_ps[:, hs_],
                                explvb_t[gg][:, col:col + 1],
                                src_[:, so], start=st, stop=sp)

                for g in range(NG):
                    hc_cp = emit_p0(g)
                    if g >= 1:
                        for tt in (2 * (g - 1), 2 * (g - 1) + 1):
                            emit_mm(tt, emit_z(tt))
                    hc_cp()
                for tt in (2 * NG - 2, 2 * NG - 1):
                    emit_mm(tt, emit_z(tt))

                # normalize and write out
                rs_sb = work.tile([1, S], dt.float32, name="rs_sb", tag="tB")
                nc.vector.tensor_scalar_add(rs_sb[:], rs_ps[:], 1e-30)
                rrec = work.tile([1, S], dt.float32, name="rrec", tag="sg")
                nc.vector.reciprocal(rrec[:], rs_sb[:])
                rbc = work.tile([128, S], dt.float32, name="rbcs", tag="u")
                for h in range(S // 512):
                    hs_ = slice(512 * h, 512 * (h + 1))
                    nc.tensor.matmul(rbc_ps[:, 0:512], ones_row[:],
                                     rrec[:, hs_], start=True, stop=True)
                    nc.scalar.copy(rbc[:, hs_], rbc_ps[:, 0:512])
                out_sb = work.tile([F, S], dt.float32, name="out_sb", tag="tA")
                nc.vector.tensor_tensor(out_sb[:], outT_ps[:], rbc[:], OP.mult)
                nc.sync.dma_start(outT.ap(), out_sb[:])

    nc.compile()
    return nc


def kernel(h_context, h_structure, edge_index, Wc_w, Wc_b, Ws_w, Ws_b,
           ac_w, as_w, Ws_coff, Wc_coff):
    from concourse.bass_utils import run_bass_kernel_spmd

    h_context = np.asarray(h_context, np.float32)
    h_structure = np.asarray(h_structure, np.float32)
    Wc_w = np.asarray(Wc_w, np.float32)
    Wc_b = np.asarray(Wc_b, np.float32)
    Ws_w = np.asarray(Ws_w, np.float32)
    Ws_b = np.asarray(Ws_b, np.float32)
    ac_w = np.asarray(ac_w, np.float32)
    as_w = np.asarray(as_w, np.float32)
    ei = np.asarray(edge_index)

    wA = float(abs(np.float32(np.asarray(Ws_coff)[0, 0])))  # scales alpha_c
    wB = float(abs(np.float32(np.asarray(Wc_coff)[0, 0])))  # scales alpha_s

    pA1 = wA * (Wc_w.T @ ac_w[0, :F])
    pA2 = wA * (Wc_w.T @ ac_w[0, F:])
    cA1 = wA * float(Wc_b @ ac_w[0, :F])
    cA2 = wA * float(Wc_b @ ac_w[0, F:])
    pB1 = wB * (Ws_w.T @ as_w[0, :F])
    pB2 = wB * (Ws_w.T @ as_w[0, F:])
    cB1 = wB * float(Ws_b @ as_w[0, :F])
    cB2 = wB * float(Ws_b @ as_w[0, F:])

    if 0 not in _BUILD_CACHE:
        _BUILD_CACHE[0] = _build_program()
    nc = _BUILD_CACHE[0]

    # adjacency, transposed + partition-major re-layout (edge -> 1)
    adjT = np.zeros((N, N), np.uint8)
    adjT[ei[1], ei[0]] = 1

    import ml_dtypes
    hctxT = np.ascontiguousarray(h_context.T)
    hstrT = np.ascontiguousarray(h_structure.T)
    hstrTe = np.exp(hstrT)
    hctxT16 = np.ascontiguousarray(hctxT.astype(ml_dtypes.bfloat16))
    hstrT16 = np.ascontiguousarray(hstrTe.astype(ml_dtypes.bfloat16))
    wub_np = np.ascontiguousarray(np.concatenate(
        [Wc_w.T, pA2[:, None], np.ones((K, 1), np.float32),
         pB2[:, None], pB1[:, None], pA1[:, None]],
        axis=1).astype(np.float32))

    # host replicas of the projections for per-core range bounds (numerical
    # shim only; the bound cancels in the softmax normalization)
    srcA = h_context @ pA1 + (cA1 + cA2)
    dstA = h_context @ pA2
    e_str = np.exp(h_structure - h_structure.max(axis=1, keepdims=True))
    sm = e_str / e_str.sum(axis=1, keepdims=True)
    srcB = sm @ pB1 + (cB1 + cB2)
    dstB = sm @ pB2
    lv_full = 0.01 * (dstA + dstB + cA2 + cB2)
    Clv = float(lv_full.max())

    dA_max = float(dstA.max())
    dB_max = float(dstB.max())

    in_maps = []
    for d in range(NC):
        sl = slice(S * d, S * (d + 1))
        mA = max(0.0, float(srcA[sl].max()) + dA_max)
        mB = max(0.0, float(srcB[sl].max()) + dB_max)
        c0 = 0.99 * (mA + mB)
        mp = adjT[:, sl].reshape(N // 128, 128, S).transpose(1, 0, 2)
        d_idx = [c for c in range(NCH) if c % 4 < 2]
        g_idx = [c for c in range(NCH) if c % 4 >= 2]
        maskPd = np.ascontiguousarray(
            mp[:, d_idx, :].reshape(128, 2 * NSLAB * S).astype(np.uint16))
        maskPg = np.ascontiguousarray(
            mp[:, g_idx, :].reshape(128, 2 * NSLAB * S))
        in_maps.append({
            "hctxT": hctxT16,
            "hstrT": hstrT16,
            "hctxT_my": np.ascontiguousarray(
                hctxT[:, sl].astype(ml_dtypes.bfloat16)),
            "hstrT_my": np.ascontiguousarray(
                hstrTe[:, sl].astype(ml_dtypes.bfloat16)),
            "wub": wub_np.astype(ml_dtypes.bfloat16),
            "maskPd": maskPd,
            "maskPg": maskPg,
            "smalls4": np.ascontiguousarray(np.broadcast_to(
                np.array([-c0, 0.01 * (cA2 + cB2) - Clv,
                          cA1 + cA2, cB1 + cB2], np.float32),
                (128, 4))),
        })

    res = run_bass_kernel_spmd(nc, in_maps, core_ids=list(range(NC)))
    out = np.empty((N, F), np.float32)
    for d in range(NC):
        out[S * d:S * (d + 1), :] = res.results[d]["outT"].T

    # hc bias: attention rows sum to 1, so + Wc_b exactly
    if np.any(Wc_b != 0.0):
        out += Wc_b[None, :]

    # rows with no edges: reference gives uniform attention = mean of hc
    row_deg = np.zeros(N, np.int64)
    np.add.at(row_deg, ei[0], 1)
    empty = row_deg == 0
    if empty.any():
        hc_host = h_context @ Wc_w.T + Wc_b
        out[empty, :] = hc_host.mean(axis=0)

    return out



# revision 53
# speedup vs baseline: 1.3926x; 1.0499x over previous
"""Trainium2 Bass kernel for nn_MixAttention (GAT-style mixed attention).

Strategy (8 cores, i-sharded over query rows, transposed compute):
  - Device computes scores in transposed layout [j on partitions, i free] so
    out^T += hc_chunk.T @ P^T_chunk contracts over partitions, no transposes.
  - lrelu decomposition: lrelu(x) = 0.01x + 0.99*relu(x). For both score
    terms the relu part is computed per entry; the linear part is rank-1:
    the per-i piece cancels in the row softmax, the per-j piece rides along
    as a multiplicative exp(lv[j]) folded into the mask op's scalar slot.
  - Per chunk: tA = relu(bcA + agrid_c) and tB = relu(bcB + bgrid_c) via
    single tensor_scalar(add, max-0) ops (fp16 packed = DVE 4x mode),
    z = tA + tB (fp16 TT, 2x), exp over half-groups of 2 chunks on ACT
    (scale=0.99, per-core bias tensor), then Pm = P * slab (plain mult;
    slab is the uint16 adjacency so masked entries are exactly 0; uint16
    keeps the DVE 2-byte fast path). exp(lv) is folded into the matmul
    stationaries instead: hc' = hc * explv (ACT copy with per-partition
    scale) and the rowsum stationary is an explv bf16 column.
  - PE accumulates out^T += hc'_c @ Pm and rowsum += explv_c @ Pm in bf16.
  - Phase-0 grids (dstA, dstB, sigma, lv) come from per-chunk PE matmuls
    into one PSUM block (129-wide psA with dstA as column F, 2-wide
    sigma/dstB pairs packed into the same banks) post-processed in batched
    [128, 8] ops. psA and sigma/dstB accumulation groups are emitted in
    disjoint time ranges: interleaving two open matmul accumulations in
    one PSUM bank corrupts results, as does issuing two matmuls on
    alternating stationaries with 1-column movings.
  - Phase 0 and phase 1 are software-pipelined: the emission loop issues
    phase-0 group g, then phase-1 groups 2(g-1) and 2(g-1)+1, then the hc
    copies of g, so every engine's in-order queue interleaves both phases.
    Grid tiles are per-group (unique names) to avoid whole-tile WAR
    hazards that would serialize the phases.
  - h inputs are bf16 (host-converted): per-query projection errors cancel
    in the row softmax; per-key errors are ~0.4% and measured end-to-end
    rel err is 4.8e-3 against the fp32 reference.
  - All per-core constants enter via input tensors (negc0/negclv/cabA/
    cabB), so a single compiled program serves every core and input set.
"""

import numpy as np

N = 8192
K = 256
F = 128
NC = 8
S = N // NC          # 1024 query rows per core
NCH = N // 128       # 64 j-chunks
KC = K // 128        # 2 contraction chunks
G0 = 8               # j-chunks per phase-0 stream group
W0 = G0 * 128
GRP = 4              # j-chunks per exp group / mask slab
NSLAB = NCH // GRP   # 16

_BUILD_CACHE = {}


def _build_program():
    import contextlib

    import concourse.bacc as bacc
    import concourse.tile as tile
    from concourse import mybir

    nc = bacc.Bacc("TRN2", target_bir_lowering=False, debug=False, num_devices=NC)
    dt = mybir.dt
    AF = mybir.ActivationFunctionType
    OP = mybir.AluOpType

    hctxT = nc.dram_tensor("hctxT", [K, N], dt.bfloat16, kind="ExternalInput")
    hstrT = nc.dram_tensor("hstrT", [K, N], dt.bfloat16, kind="ExternalInput")
    hctxT_my = nc.dram_tensor("hctxT_my", [K, S], dt.bfloat16,
                              kind="ExternalInput")
    hstrT_my = nc.dram_tensor("hstrT_my", [K, S], dt.bfloat16,
                              kind="ExternalInput")
    # wub = [wpack (F+1) | ones,pB2,pB1 (3) | pA1 (1)]: one DMA per k-chunk
    wub = nc.dram_tensor("wub", [K, F + 5], dt.bfloat16, kind="ExternalInput")
    maskPd = nc.dram_tensor("maskPd", [128, 2 * NSLAB * S], dt.uint16,
                            kind="ExternalInput")
    maskPg = nc.dram_tensor("maskPg", [128, 2 * NSLAB * S], dt.uint8,
                            kind="ExternalInput")
    smalls4 = nc.dram_tensor("smalls4", [128, 4], dt.float32,
                             kind="ExternalInput")
    outT = nc.dram_tensor("outT", [F, S], dt.float32, kind="ExternalOutput")

    with tile.TileContext(nc) as tc:
        with contextlib.ExitStack() as ctx:
            vecs = ctx.enter_context(tc.tile_pool(name="vecs", bufs=1))
            hcpool = ctx.enter_context(tc.tile_pool(name="hc", bufs=1))
            stp = ctx.enter_context(tc.tile_pool(name="stream", bufs=2))
            work = ctx.enter_context(tc.tile_pool(name="work", bufs=3))
            grpp = ctx.enter_context(tc.tile_pool(name="grp", bufs=3))
            pmp = ctx.enter_context(tc.tile_pool(name="pm", bufs=6))
            slabp = ctx.enter_context(tc.tile_pool(name="slabp", bufs=3))

            # ---- small inputs (3 + 4 DMAs) ----
            sm4_sb = vecs.tile([128, 4], dt.float32, name="sm4_sb")
            nc.sync.dma_start(sm4_sb[:], smalls4.ap())
            negc0_sb = sm4_sb[:, 0:1]
            negclv_sb = sm4_sb[:, 1:2]
            cabA_sb = sm4_sb[:, 2:3]
            cabB_sb = sm4_sb[:, 3:4]
            wub_sb = [vecs.tile([128, F + 5], dt.bfloat16, name=f"wub{k}")
                      for k in range(KC)]
            my_str = [stp.tile([128, S], dt.bfloat16, name=f"mystr{k}",
                               tag=f"hst{k}", bufs=3) for k in range(KC)]
            my_ctx = [stp.tile([128, S], dt.bfloat16, name=f"myctx{k}",
                               tag=f"hct{k}", bufs=3) for k in range(KC)]
            for k in range(KC):
                ks = slice(128 * k, 128 * (k + 1))
                nc.sync.dma_start(wub_sb[k][:], wub.ap()[ks, :])
                nc.sync.dma_start(my_str[k][:], hstrT_my.ap()[ks, :])
                nc.sync.dma_start(my_ctx[k][:], hctxT_my.ap()[ks, :])

            # ---- src rows for my i-slice ----
            sigrow = work.tile([1, S], dt.float32, name="sigrow", tag="u")
            srcArow = work.tile([1, S], dt.float32, name="srcArow", tag="tB")
            srcBrow = work.tile([1, S], dt.float32, name="srcBrow", tag="tA")
            with tc.tile_pool(name="psrow", bufs=1, space="PSUM") as psrow:
                psr0 = psrow.tile([1, S], dt.float32, name="psr0")
                psr1 = psrow.tile([1, S], dt.float32, name="psr1")
                psra = psrow.tile([1, S], dt.float32, name="psra")
                for k in range(KC):
                    st, sp = (k == 0), (k == KC - 1)
                    for h in range(S // 512):
                        hs_ = slice(512 * h, 512 * (h + 1))
                        nc.tensor.matmul(psr0[:, hs_], wub_sb[k][:, F + 1:F + 2],
                                         my_str[k][:, hs_], start=st, stop=sp)
                        nc.tensor.matmul(psr1[:, hs_], wub_sb[k][:, F + 3:F + 4],
                                         my_str[k][:, hs_], start=st, stop=sp)
                        nc.tensor.matmul(psra[:, hs_], wub_sb[k][:, F + 4:F + 5],
                                         my_ctx[k][:, hs_], start=st, stop=sp)
                nc.vector.reciprocal(sigrow[:], psr0[:])
                nc.scalar.copy(srcArow[:], psra[:])
                nc.vector.tensor_tensor(srcBrow[:], psr1[:], sigrow[:], OP.mult)

            ones_row = vecs.tile([1, 128], dt.float32, name="ones_row")
            nc.vector.memset(ones_row[:], 1.0)

            # broadcast rows -> [128, S] fp16 tiles with constants folded in
            bcA = vecs.tile([128, S], dt.float16, name="bcA")
            bcB = vecs.tile([128, S], dt.float16, name="bcB")
            with tc.tile_pool(name="ps0c", bufs=1, space="PSUM") as ps0c:
                psbc = ps0c.tile([128, S], dt.float32, name="psbc")
                psbc2 = ps0c.tile([128, S], dt.float32, name="psbc2")
                for h in range(S // 512):
                    hs_ = slice(512 * h, 512 * (h + 1))
                    nc.tensor.matmul(psbc[:, hs_], ones_row[:], srcArow[:, hs_],
                                     start=True, stop=True)
                    nc.tensor.matmul(psbc2[:, hs_], ones_row[:], srcBrow[:, hs_],
                                     start=True, stop=True)
                nc.vector.tensor_scalar(bcA[:], psbc[:], cabA_sb, None, OP.add)
                nc.vector.tensor_scalar(bcB[:], psbc2[:], cabB_sb, None, OP.add)

            # ---- phase 0/1 software-pipelined ----
            # per-group grid tiles (unique names: no cross-phase WAR hazards)
            NG = NCH // G0
            agrid_t = [vecs.tile([128, G0], dt.float32, name=f"ag{g}")
                       for g in range(NG)]
            bgrid_t = [vecs.tile([128, G0], dt.float32, name=f"bg{g}")
                       for g in range(NG)]
            explv_t = [vecs.tile([128, G0], dt.float32, name=f"lv{g}")
                       for g in range(NG)]
            explvb_t = [vecs.tile([128, G0], dt.bfloat16, name=f"lvb{g}")
                        for g in range(NG)]
            hc_sb = [hcpool.tile([128, F], dt.bfloat16, name=f"hc{c}")
                     for c in range(NCH)]
            slabs = []
            AST = 171  # psA chunk stride (fp32 elems); no PSUM bank crossings

            with contextlib.ExitStack() as pctx:
                ps1 = pctx.enter_context(
                    tc.tile_pool(name="ps1", bufs=1, space="PSUM"))
                ps0 = pctx.enter_context(
                    tc.tile_pool(name="ps0", bufs=1, space="PSUM"))
                outT_ps = ps1.tile([F, S], dt.float32, name="outT_ps")
                rs_ps = ps1.tile([1, S], dt.float32, name="rs_ps")
                rbc_ps = ps1.tile([128, S // 2], dt.float32, name="rbc_ps")

                def emit_p0(g):
                    t0 = 2 * g
                    slab_d = slabp.tile([128, 4 * S], dt.uint16,
                                        name="slabd", bufs=2)
                    nc.sync.dma_start(
                        slab_d[:],
                        maskPd.ap()[:, 2 * t0 * S:2 * (t0 + 2) * S])
                    slab_g = slabp.tile([128, 4 * S], dt.uint8,
                                        name="slabg", bufs=2)
                    nc.sync.dma_start(
                        slab_g[:],
                        maskPg.ap()[:, 2 * t0 * S:2 * (t0 + 2) * S])
                    slabs.append((slab_d[:, 0:2 * S], slab_g[:, 0:2 * S]))
                    slabs.append((slab_d[:, 2 * S:4 * S],
                                  slab_g[:, 2 * S:4 * S]))
                    gs = slice(W0 * g, W0 * (g + 1))
                    hst = [stp.tile([128, W0], dt.bfloat16, name=f"hstg{k}",
                                    tag=f"hst{k}", bufs=3) for k in range(KC)]
                    hct = [stp.tile([128, W0], dt.bfloat16, name=f"hctg{k}",
                                    tag=f"hct{k}", bufs=3) for k in range(KC)]
                    for k in range(KC):
                        ks = slice(128 * k, 128 * (k + 1))
                        nc.sync.dma_start(hst[k][:], hstrT.ap()[ks, gs])
                        nc.sync.dma_start(hct[k][:], hctxT.ap()[ks, gs])
                    # one matmul per stationary load (129/2-col movings);
                    # dstA rides as column F of the psA block
                    psAg = ps0.tile([128, AST * G0 + 2 * G0], dt.float32,
                                    name="psAg")
                    SBO = AST * G0
                    # NOTE: keep the psA and psSB accumulation groups in
                    # separate time ranges — interleaving two open matmul
                    # accumulations in one PSUM bank corrupts the results.
                    for cc in range(G0):
                        cs = slice(128 * cc, 128 * (cc + 1))
                        for k in range(KC):
                            st, sp = (k == 0), (k == KC - 1)
                            nc.tensor.matmul(
                                psAg[:, SBO + 2 * cc:SBO + 2 * cc + 2],
                                hst[k][:, cs],
                                wub_sb[k][:, F + 1:F + 3], start=st, stop=sp)
                    for cc in range(G0):
                        cs = slice(128 * cc, 128 * (cc + 1))
                        for k in range(KC):
                            st, sp = (k == 0), (k == KC - 1)
                            nc.tensor.matmul(
                                psAg[:, AST * cc:AST * cc + F + 1],
                                hct[k][:, cs], wub_sb[k][:, 0:F + 1],
                                start=st, stop=sp)
                    # batched grid math for this group of 8 chunks
                    sg = work.tile([128, G0], dt.float32, name="sg", tag="sg")
                    nc.vector.reciprocal(sg[:],
                                         psAg[:, SBO:SBO + 2 * G0:2])
                    nc.vector.tensor_tensor(bgrid_t[g][:],
                                            psAg[:, SBO + 1:SBO + 2 * G0:2],
                                            sg[:], OP.mult)
                    nc.vector.tensor_copy(agrid_t[g][:],
                                          psAg[:, F:AST * G0:AST])
                    lvt = work.tile([128, G0], dt.float32, name="lvt", tag="sg")
                    nc.vector.tensor_tensor(lvt[:], agrid_t[g][:],
                                            bgrid_t[g][:], OP.add)
                    nc.scalar.activation(explv_t[g][:], lvt[:], AF.Exp,
                                         bias=negclv_sb, scale=0.01)
                    nc.scalar.copy(explvb_t[g][:], explv_t[g][:])
                    # single strided copy off PSUM frees psAg quickly (short
                    # phase-0 spine); the per-chunk explv scaling happens
                    # off-spine from SBUF
                    hcraw = work.tile([128, G0 * F], dt.bfloat16,
                                      name="hcraw", tag="hcraw", bufs=2)
                    src3 = psAg[:, 0:AST * G0].rearrange(
                        "p (g a) -> p g a", a=AST)[:, :, 0:F]
                    dst3 = hcraw[:].rearrange("p (g f) -> p g f", f=F)
                    nc.scalar.copy(dst3, src3)

                    def hc_copies(g=g, hcraw=hcraw):
                        # hc' = hc * explv[j] (per-partition ACT-copy scale)
                        for cc in range(G0):
                            c = G0 * g + cc
                            nc.scalar.mul(hc_sb[c][:],
                                          hcraw[:, F * cc:F * (cc + 1)],
                                          explv_t[g][:, cc:cc + 1])
                    return hc_copies

                def emit_z(t):
                    zgrp = grpp.tile([128, GRP * S], dt.float16, name="zgrp")
                    Pgrp = grpp.tile([128, GRP * S], dt.bfloat16, name="Pgrp")
                    for pair in range(GRP // 2):
                        tAt = work.tile([128, 2 * S], dt.float16, name="tA",
                                        tag="tA")
                        tBt = work.tile([128, 2 * S], dt.float16, name="tB",
                                        tag="tB")
                        for h in range(2):
                            cc = 2 * pair + h
                            c = t * GRP + cc
                            gg, col = c // G0, c % G0
                            hs_ = slice(h * S, (h + 1) * S)
                            nc.vector.tensor_scalar(
                                tAt[:, hs_], bcA[:],
                                agrid_t[gg][:, col:col + 1],
                                0.0, OP.add, OP.max)
                            nc.vector.tensor_scalar(
                                tBt[:, hs_], bcB[:],
                                bgrid_t[gg][:, col:col + 1],
                                0.0, OP.add, OP.max)
                        o = pair * 2 * S
                        nc.vector.tensor_tensor(zgrp[:, o:o + 2 * S], tAt[:],
                                                tBt[:], OP.add)
                    nc.scalar.activation(Pgrp[:], zgrp[:], AF.Exp,
                                         bias=negc0_sb, scale=0.99)
                    return Pgrp

                def emit_mm(t, Pgrp):
                    slab_d, slab_g = slabs[t]
                    Pmd = pmp.tile([128, 2 * S], dt.bfloat16, name="Pmd",
                                   tag="pmd", bufs=3)
                    Pmg = pmp.tile([128, 2 * S], dt.bfloat16, name="Pmg",
                                   tag="pmg", bufs=3)
                    nc.vector.tensor_tensor(Pmd[:], Pgrp[:, 0:2 * S],
                                            slab_d, OP.mult)
                    nc.gpsimd.tensor_tensor(Pmg[:], Pgrp[:, 2 * S:4 * S],
                                            slab_g, OP.mult)
                    for cc in range(GRP):
                        c = t * GRP + cc
                        gg, col = c // G0, c % G0
                        src_ = Pmd if cc < 2 else Pmg
                        o = (cc % 2) * S
                        st = (c == 0)
                        sp = (c == NCH - 1)
                        for h in range(S // 512):
                            hs_ = slice(512 * h, 512 * (h + 1))
                            so = slice(o + 512 * h, o + 512 * (h + 1))
                            nc.tensor.matmul(outT_ps[:, hs_], hc_sb[c][:],
                                             src_[:, so], start=st, stop=sp)
                            nc.tensor.matmul(
                                rs_ps[:, hs_],
                                explvb_t[gg][:, col:col + 1],
                                src_[:, so], start=st, stop=sp)

                for g in range(NG):
                    hc_cp = emit_p0(g)
                    if g >= 1:
                        for tt in (2 * (g - 1), 2 * (g - 1) + 1):
                            emit_mm(tt, emit_z(tt))
                    hc_cp()
                for tt in (2 * NG - 2, 2 * NG - 1):
                    emit_mm(tt, emit_z(tt))

                # normalize and write out
                rs_sb = work.tile([1, S], dt.float32, name="rs_sb", tag="tB")
                nc.vector.tensor_scalar_add(rs_sb[:], rs_ps[:], 1e-30)
                rrec = work.tile([1, S], dt.float32, name="rrec", tag="sg")
                nc.vector.reciprocal(rrec[:], rs_sb[:])
                rbc = work.tile([128, S], dt.float32, name="rbcs", tag="u")
                for h in range(S // 512):
                    hs_ = slice(512 * h, 512 * (h + 1))
                    nc.tensor.matmul(rbc_ps[:, 0:512], ones_row[:],
                                     rrec[:, hs_], start=True, stop=True)
                    nc.scalar.copy(rbc[:, hs_], rbc_ps[:, 0:512])
                out_sb = work.tile([F, S], dt.float32, name="out_sb", tag="tA")
                nc.vector.tensor_tensor(out_sb[:], outT_ps[:], rbc[:], OP.mult)
                nc.sync.dma_start(outT.ap(), out_sb[:])

    nc.compile()
    return nc


def kernel(h_context, h_structure, edge_index, Wc_w, Wc_b, Ws_w, Ws_b,
           ac_w, as_w, Ws_coff, Wc_coff):
    from concourse.bass_utils import run_bass_kernel_spmd

    h_context = np.asarray(h_context, np.float32)
    h_structure = np.asarray(h_structure, np.float32)
    Wc_w = np.asarray(Wc_w, np.float32)
    Wc_b = np.asarray(Wc_b, np.float32)
    Ws_w = np.asarray(Ws_w, np.float32)
    Ws_b = np.asarray(Ws_b, np.float32)
    ac_w = np.asarray(ac_w, np.float32)
    as_w = np.asarray(as_w, np.float32)
    ei = np.asarray(edge_index)

    wA = float(abs(np.float32(np.asarray(Ws_coff)[0, 0])))  # scales alpha_c
    wB = float(abs(np.float32(np.asarray(Wc_coff)[0, 0])))  # scales alpha_s

    pA1 = wA * (Wc_w.T @ ac_w[0, :F])
    pA2 = wA * (Wc_w.T @ ac_w[0, F:])
    cA1 = wA * float(Wc_b @ ac_w[0, :F])
    cA2 = wA * float(Wc_b @ ac_w[0, F:])
    pB1 = wB * (Ws_w.T @ as_w[0, :F])
    pB2 = wB * (Ws_w.T @ as_w[0, F:])
    cB1 = wB * float(Ws_b @ as_w[0, :F])
    cB2 = wB * float(Ws_b @ as_w[0, F:])

    if 0 not in _BUILD_CACHE:
        _BUILD_CACHE[0] = _build_program()
    nc = _BUILD_CACHE[0]

    # adjacency, transposed + partition-major re-layout (edge -> 1)
    adjT = np.zeros((N, N), np.uint8)
    adjT[ei[1], ei[0]] = 1

    import ml_dtypes
    hctxT = np.ascontiguousarray(h_context.T)
    hstrT = np.ascontiguousarray(h_structure.T)
    hstrTe = np.exp(hstrT)
    hctxT16 = np.ascontiguousarray(hctxT.astype(ml_dtypes.bfloat16))
    hstrT16 = np.ascontiguousarray(hstrTe.astype(ml_dtypes.bfloat16))
    wub_np = np.ascontiguousarray(np.concatenate(
        [Wc_w.T, pA2[:, None], np.ones((K, 1), np.float32),
         pB2[:, None], pB1[:, None], pA1[:, None]],
        axis=1).astype(np.float32))

    # host replicas of the projections for per-core range bounds (numerical
    # shim only; the bound cancels in the softmax normalization)
    srcA = h_context @ pA1 + (cA1 + cA2)
    dstA = h_context @ pA2
    e_str = np.exp(h_structure - h_structure.max(axis=1, keepdims=True))
    sm = e_str / e_str.sum(axis=1, keepdims=True)
    srcB = sm @ pB1 + (cB1 + cB2)
    dstB = sm @ pB2
    lv_full = 0.01 * (dstA + dstB + cA2 + cB2)
    Clv = float(lv_full.max())

    dA_max = float(dstA.max())
    dB_max = float(dstB.max())

    in_maps = []
    for d in range(NC):
        sl = slice(S * d, S * (d + 1))
        mA = max(0.0, float(srcA[sl].max()) + dA_max)
        mB = max(0.0, float(srcB[sl].max()) + dB_max)
        c0 = 0.99 * (mA + mB)
        mp = adjT[:, sl].reshape(N // 128, 128, S).transpose(1, 0, 2)
        d_idx = [c for c in range(NCH) if c % 4 < 2]
        g_idx = [c for c in range(NCH) if c % 4 >= 2]
        maskPd = np.ascontiguousarray(
            mp[:, d_idx, :].reshape(128, 2 * NSLAB * S).astype(np.uint16))
        maskPg = np.ascontiguousarray(
            mp[:, g_idx, :].reshape(128, 2 * NSLAB * S))
        in_maps.append({
            "hctxT": hctxT16,
            "hstrT": hstrT16,
            "hctxT_my": np.ascontiguousarray(
                hctxT[:, sl].astype(ml_dtypes.bfloat16)),
            "hstrT_my": np.ascontiguousarray(
                hstrTe[:, sl].astype(ml_dtypes.bfloat16)),
            "wub": wub_np.astype(ml_dtypes.bfloat16),
            "maskPd": maskPd,
            "maskPg": maskPg,
            "smalls4": np.ascontiguousarray(np.broadcast_to(
                np.array([-c0, 0.01 * (cA2 + cB2) - Clv,
                          cA1 + cA2, cB1 + cB2], np.float32),
                (128, 4))),
        })

    res = run_bass_kernel_spmd(nc, in_maps, core_ids=list(range(NC)))
    out = np.empty((N, F), np.float32)
    for d in range(NC):
        out[S * d:S * (d + 1), :] = res.results[d]["outT"].T

    # hc bias: attention rows sum to 1, so + Wc_b exactly
    if np.any(Wc_b != 0.0):
        out += Wc_b[None, :]

    # rows with no edges: reference gives uniform attention = mean of hc
    row_deg = np.zeros(N, np.int64)
    np.add.at(row_deg, ei[0], 1)
    empty = row_deg == 0
    if empty.any():
        hc_host = h_context @ Wc_w.T + Wc_b
        out[empty, :] = hc_host.mean(axis=0)

    return out

